# revision 5
# baseline (speedup 1.0000x reference)
"""BindingPocketGNN (3-layer GCN, N=50000, E=800000) on 8 Trainium2 NeuronCores.

Distribution: nodes sharded into 8 contiguous ranges (6250/core). Each core owns the
scatter/aggregation for its destination-node range; edges are routed (host-side) to the
core owning their destination. Source features come from a replicated node-major table
(input x for layer 1; AllGather-replicated activations for layers 2/3).

v2: the per-edge gather uses a few big gpsimd dma_gather (InstDMAGatherAnt) calls per
layer (0.34ns/descriptor batched SWDGE) instead of ~900 small indirect_dma_start ops
(~1us fixed each, which made the baseline SWDGE/gpsimd-bound at 87% occupancy).
dma_gather takes int16 indices, so the 50000-row table is addressed as two halves
(src < 32768 via table[0:32768], src >= 32768 via table[32768:]); edges are grouped
host-side by (dst tile, half) into 128-edge subtiles, padded with idx=0 slots that a
one-hot mask zeroes out.

Per layer, on each core (feat-major formulation so BN/bias are per-partition):
    msg[e, f]     = table[src_e, f]                      (chunked dma_gather, bf16)
    mask[e, d]    = (iota[d]==dloc[e]) * norm_e          (DVE tensor_scalar, one op)
    z^T[f, d]    += msg^T @ mask                         (TensorE accumulate over subtiles)
    y^T[f_out, d] = W^T @ z^T                            (TensorE)
    stats         = AllReduce(sum/sumsq of y)            (1KB collective; BN layers)
    act^T         = Relu(A*y^T + B)                      (ScalarE, per-partition A/B)
    h             = act^T transposed to node-major       (TensorE transpose)
    table_{l+1}   = AllGather(h)                         (collective; layers 1,2)
Layer 3 feeds a [128->1] FC matmul; +fcb and sigmoid applied on host.

norm_e = dinv[src]*dinv[dst] is folded into the mask values (host-computed fp32,
rounded to bf16); tables hold raw (unscaled) features.
"""
import sys
if "/opt/trn_rl_repo" not in sys.path:
    sys.path.insert(0, "/opt/trn_rl_repo")

import os
import numpy as np
import ml_dtypes

import concourse.bass as bass
import concourse.bacc as bacc
import concourse.mybir as mybir
import concourse.tile as tile
from concourse import bass_utils
from concourse.masks import make_identity

N = 50000
E = 800000
IN, HID = 64, 128
BN_EPS = 1e-5
NCORES = 8
NPC = N // NCORES          # 6250 nodes per core
P = 128
NT = (NPC + P - 1) // P    # 49 dst tiles per core
LAST_D = NPC - (NT - 1) * P  # 106
SPLIT = 32768              # int16 index range boundary for dma_gather

F32 = mybir.dt.float32
I16 = mybir.dt.int16
BF16 = mybir.dt.bfloat16
DT_TAB = BF16
NP_TAB = np.dtype(ml_dtypes.bfloat16)

REPS = int(os.environ.get("GCN_REPS", "1"))
GMAX = int(os.environ.get("GCN_GMAX", "72"))   # max subtiles per dma_gather chunk
LAYERS = int(os.environ.get("GCN_LAYERS", "3"))  # debug: run only first k layers

Alu = mybir.AluOpType
Act = mybir.ActivationFunctionType

_NC_CACHE = {}


def _chunk_plan(S2):
    """Greedy-pack tiles' subtile groups into gather chunks of <= GMAX subtiles.
    Returns (chunks, per_tile): chunks[h] = list of (sub_lo_in_half, n_sub);
    per_tile[h][t] = (chunk_id, col0) locating tile t's first subtile of half h."""
    chunks, per_tile = [], []
    for h in (0, 1):
        ch, pt = [], []
        lo, n = 0, 0
        for t in range(NT):
            s = S2[t][h]
            if n + s > GMAX and n > 0:
                ch.append((lo, n))
                lo += n
                n = 0
            pt.append((len(ch), n))
            n += s
        if n:
            ch.append((lo, n))
        chunks.append(ch)
        per_tile.append(pt)
    return chunks, per_tile


def _build(S2):
    """Build+schedule the SPMD program. S2 = tuple of (S_low, S_high) per dst tile;
    identical for all 8 cores."""
    S2 = [tuple(s) for s in S2]
    T_L = sum(s[0] for s in S2)
    T_H = sum(s[1] for s in S2)
    TT = T_L + T_H
    chunks, per_tile = _chunk_plan(S2)
    # global subtile index (dloc/norm column): low half at sub_lo, high at T_L + sub_lo
    half_base = (0, T_L)

    nc = bacc.Bacc("TRN2", target_bir_lowering=False, debug=False, num_devices=NCORES)

    # ---- I/O ----
    xs = nc.dram_tensor("xs", [N, HID], DT_TAB, kind="ExternalInput")
    gidx_d = nc.dram_tensor("gidx", [P, TT * 8], I16, kind="ExternalInput")
    dloc_d = nc.dram_tensor("dloc", [P, TT], F32, kind="ExternalInput")
    nrm_d = nc.dram_tensor("nrm", [P, TT], F32, kind="ExternalInput")
    W_d = [
        nc.dram_tensor("W1", [IN, HID], F32, kind="ExternalInput"),
        nc.dram_tensor("W2", [HID, HID], F32, kind="ExternalInput"),
        nc.dram_tensor("W3", [HID, HID], F32, kind="ExternalInput"),
    ]
    fcW_d = nc.dram_tensor("fcW", [HID, 1], F32, kind="ExternalInput")
    g_d = [nc.dram_tensor("g1", [HID, 1], F32, kind="ExternalInput"),
           nc.dram_tensor("g2", [HID, 1], F32, kind="ExternalInput")]
    bt_d = [nc.dram_tensor("bt1", [HID, 1], F32, kind="ExternalInput"),
            nc.dram_tensor("bt2", [HID, 1], F32, kind="ExternalInput")]
    b3_d = nc.dram_tensor("b3", [HID, 1], F32, kind="ExternalInput")
    outv = nc.dram_tensor("outv", [1, NPC], F32, kind="ExternalOutput")

    with tile.TileContext(nc) as tc:
        with (
            tc.tile_pool(name="meta", bufs=1) as meta,
            tc.tile_pool(name="msgLp", bufs=2) as msgLp,
            tc.tile_pool(name="msgHp", bufs=2) as msgHp,
            tc.tile_pool(name="maskp", bufs=8) as maskp,
            tc.tile_pool(name="zsp", bufs=3) as zsp,
            tc.tile_pool(name="actp", bufs=3) as actp,
            tc.tile_pool(name="hp", bufs=3) as hp,
            tc.tile_pool(name="sqp", bufs=2) as sqp,
            tc.tile_pool(name="zps_p", bufs=2, space="PSUM") as zps_p,
            tc.tile_pool(name="yps_p", bufs=2, space="PSUM") as yps_p,
            tc.tile_pool(name="trps_p", bufs=2, space="PSUM") as trps_p,
            tc.tile_pool(name="fcps_p", bufs=1, space="PSUM") as fcps_p,
            tc.tile_pool(name="dram", bufs=1, space="DRAM") as dram,
        ):
            # ---- resident metadata ----
            gidx_sb = meta.tile([P, TT * 8], I16)
            nc.sync.dma_start(gidx_sb[:], gidx_d[:])
            dloc_sb = meta.tile([P, TT], F32)
            nc.sync.dma_start(dloc_sb[:], dloc_d[:])
            nrm_sb = meta.tile([P, TT], F32)
            nc.sync.dma_start(nrm_sb[:], nrm_d[:])
            W_sb = []
            for l in range(3):
                fi = IN if l == 0 else HID
                w = meta.tile([fi, HID], F32, name=f"W{l}_sb")
                nc.sync.dma_start(w[:], W_d[l][:])
                W_sb.append(w)
            fcW_sb = meta.tile([HID, 1], F32)
            nc.sync.dma_start(fcW_sb[:], fcW_d[:])
            g_sb, bt_sb = [], []
            for l in range(2):
                gg = meta.tile([HID, 1], F32, name=f"g{l}_sb")
                nc.sync.dma_start(gg[:], g_d[l][:])
                g_sb.append(gg)
                bb = meta.tile([HID, 1], F32, name=f"bt{l}_sb")
                nc.sync.dma_start(bb[:], bt_d[l][:])
                bt_sb.append(bb)
            b3_sb = meta.tile([HID, 1], F32)
            nc.sync.dma_start(b3_sb[:], b3_d[:])
            eps_sb = meta.tile([P, 1], F32)
            nc.vector.memset(eps_sb[:], BN_EPS)

            ident = meta.tile([P, P], F32)
            make_identity(nc, ident[:])
            iota_i = meta.tile([P, P], mybir.dt.int32)
            nc.gpsimd.iota(iota_i[:], pattern=[[1, P]], base=0, channel_multiplier=0)
            iota_t = meta.tile([P, P], DT_TAB)
            nc.vector.tensor_copy(iota_t[:], iota_i[:])

            ystore = meta.tile([P, NT * P], F32)
            sums = meta.tile([P, NT], F32)
            sumsq = meta.tile([P, NT], F32)
            out_store = meta.tile([1, NPC], F32)

            # internal DRAM for collectives (fresh per rep: Shared tensors allow one writer)
            def mk_coll(rep):
                tab_in = [dram.tile([NPC, HID], DT_TAB, name=f"tab{l}_in_r{rep}") for l in (1, 2)]
                tab_out = [dram.tile([N, HID], DT_TAB, name=f"tab{l}_out_r{rep}",
                                     addr_space="Shared") for l in (1, 2)]
                st_in = [dram.tile([P, 2], F32, name=f"st{l}_in_r{rep}") for l in (0, 1)]
                st_out = [dram.tile([P, 2], F32, name=f"st{l}_out_r{rep}", addr_space="Shared")
                          for l in (0, 1)]
                return tab_in, tab_out, st_in, st_out

            for _rep in range(REPS):
              tab_in, tab_out, st_in, st_out = mk_coll(_rep)
              for l in range(LAYERS):
                 f_in = IN if l == 0 else HID
                 table = xs if l == 0 else tab_out[l - 1]

                 # ---- chunked gathers (issued lazily, consumed tile-major) ----
                 chunk_tiles = {}

                 def get_chunk(h, cid, l=l, table=table, chunk_tiles=chunk_tiles):
                     key = (h, cid)
                     if key not in chunk_tiles:
                         sub_lo, nsub = chunks[h][cid]
                         pool = msgLp if h == 0 else msgHp
                         buf = pool.tile([P, GMAX * HID], DT_TAB, tag=f"msg{h}")
                         src_ap = table[0:SPLIT, :] if h == 0 else table[SPLIT:N, :]
                         g0 = half_base[h] + sub_lo
                         nidx = nsub * P
                         nc.gpsimd.dma_gather(
                             buf[:, :nsub * HID].rearrange("p (g e) -> p g e", e=HID),
                             src_ap,
                             gidx_sb[:, g0 * 8:(g0 + nsub) * 8],
                             nidx, nidx, HID, queue_num=0, single_packet=False,
                         )
                         chunk_tiles[key] = buf
                     return chunk_tiles[key]

                 # ---- aggregation + weight matmul ----
                 for t in range(NT):
                     d_hi = LAST_D if t == NT - 1 else P
                     segs = []
                     for h in (0, 1):
                         sth = S2[t][h]
                         if sth:
                             segs.append((h, per_tile[h][t][0], per_tile[h][t][1], sth))
                     ntot = sum(s[3] for s in segs)
                     zps = zps_p.tile([P, P], F32, tag="zps")
                     k = 0
                     for (h, cid, col0, sth) in segs:
                         buf = get_chunk(h, cid)
                         g0 = half_base[h] + chunks[h][cid][0] + col0
                         for s in range(sth):
                             g = g0 + s
                             mask = maskp.tile([P, P], DT_TAB, tag="mask")
                             nc.vector.tensor_scalar(
                                 out=mask[:], in0=iota_t[:],
                                 scalar1=dloc_sb[:, g:g + 1], scalar2=nrm_sb[:, g:g + 1],
                                 op0=Alu.is_equal, op1=Alu.mult,
                             )
                             nc.tensor.matmul(
                                 zps[:f_in, :],
                                 lhsT=buf[:, (col0 + s) * HID:(col0 + s) * HID + f_in],
                                 rhs=mask[:],
                                 start=(k == 0), stop=(k == ntot - 1),
                             )
                             k += 1
                     zs = zsp.tile([P, P], F32, tag="zs")
                     nc.scalar.activation(out=zs[:f_in, :], in_=zps[:f_in, :], func=Act.Copy)
                     yps = yps_p.tile([P, P], F32, tag="yps")
                     nc.tensor.matmul(yps[:], lhsT=W_sb[l][:], rhs=zs[:f_in, :],
                                      start=True, stop=True)
                     if l < 2:
                         nc.scalar.activation(
                             out=ystore[:, t * P:t * P + d_hi], in_=yps[:, :d_hi],
                             func=Act.Copy, accum_out=sums[:, t:t + 1],
                         )
                         sq = sqp.tile([P, P], F32, tag="sq")
                         nc.scalar.activation(
                             out=sq[:, :d_hi], in_=yps[:, :d_hi],
                             func=Act.Square, accum_out=sumsq[:, t:t + 1],
                         )
                     else:
                         act3 = actp.tile([P, P], F32, tag="act")
                         nc.scalar.activation(out=act3[:, :d_hi], in_=yps[:, :d_hi],
                                              func=Act.Relu, bias=b3_sb[:], scale=1.0)
                         fcp = fcps_p.tile([1, P], F32, tag="fcp")
                         nc.tensor.matmul(fcp[:1, :d_hi], lhsT=fcW_sb[:], rhs=act3[:, :d_hi],
                                          start=True, stop=True)
                         nc.vector.tensor_copy(out_store[:1, t * P:t * P + d_hi], fcp[:1, :d_hi])

                 if l == LAYERS - 1 and l < 2:
                     nc.vector.tensor_copy(out_store[:1, :], ystore[:1, :NPC])
                     break
                 if l < 2:
                     # ---- BN stats allreduce + coefficients ----
                     stats = meta.tile([P, 2], F32, name=f"stats{l}_r{_rep}")
                     nc.vector.tensor_reduce(stats[:, 0:1], sums[:], axis=mybir.AxisListType.X, op=Alu.add)
                     nc.vector.tensor_reduce(stats[:, 1:2], sumsq[:], axis=mybir.AxisListType.X, op=Alu.add)
                     nc.sync.dma_start(st_in[l][:], stats[:])
                     nc.gpsimd.collective_compute(
                         "AllReduce", Alu.add, replica_groups=[list(range(NCORES))],
                         ins=[st_in[l][:]], outs=[st_out[l][:]],
                     )
                     tot = meta.tile([P, 2], F32, name=f"tot{l}_r{_rep}")
                     nc.sync.dma_start(tot[:], st_out[l][:])
                     cf = meta.tile([P, 6], F32, name=f"cf{l}_r{_rep}")  # mean ex2 var std A B
                     nc.vector.tensor_scalar_mul(cf[:, 0:1], tot[:, 0:1], 1.0 / N)
                     nc.vector.tensor_scalar_mul(cf[:, 1:2], tot[:, 1:2], 1.0 / N)
                     nc.vector.tensor_tensor(out=cf[:, 2:3], in0=cf[:, 0:1], in1=cf[:, 0:1], op=Alu.mult)
                     nc.vector.tensor_tensor(out=cf[:, 2:3], in0=cf[:, 1:2], in1=cf[:, 2:3], op=Alu.subtract)
                     nc.scalar.activation(out=cf[:, 3:4], in_=cf[:, 2:3], func=Act.Sqrt, bias=eps_sb[:], scale=1.0)
                     nc.vector.reciprocal(cf[:, 4:5], cf[:, 3:4])
                     A = meta.tile([P, 1], F32, name=f"A{l}_r{_rep}")
                     B = meta.tile([P, 1], F32, name=f"B{l}_r{_rep}")
                     nc.vector.tensor_tensor(out=A[:], in0=g_sb[l][:], in1=cf[:, 4:5], op=Alu.mult)
                     nc.vector.tensor_tensor(out=cf[:, 5:6], in0=cf[:, 0:1], in1=A[:], op=Alu.mult)
                     nc.vector.tensor_tensor(out=B[:], in0=bt_sb[l][:], in1=cf[:, 5:6], op=Alu.subtract)

                     # ---- epilogue: act, transpose to node-major, store table slice ----
                     for t in range(NT):
                         d_hi = LAST_D if t == NT - 1 else P
                         act = actp.tile([P, P], F32, tag="act")
                         nc.scalar.activation(out=act[:, :d_hi], in_=ystore[:, t * P:t * P + d_hi],
                                              func=Act.Relu, bias=B[:], scale=A[:])
                         tr = trps_p.tile([P, P], F32, tag="tr")
                         nc.tensor.transpose(tr[:d_hi, :], act[:, :d_hi], ident[:])
                         h = hp.tile([P, HID], DT_TAB, tag="h")
                         nc.vector.tensor_copy(h[:d_hi, :], tr[:d_hi, :])
                         nc.sync.dma_start(tab_in[l][t * P:t * P + d_hi, :], h[:d_hi, :])
                     nc.gpsimd.collective_compute(
                         "AllGather", Alu.bypass, replica_groups=[list(range(NCORES))],
                         ins=[tab_in[l][:]], outs=[tab_out[l][:]],
                     )

            nc.sync.dma_start(outv[:], out_store[:])

    nc.compile()
    return nc


def _prep(inputs):
    x = np.asarray(inputs["x"], np.float32)
    ei = np.asarray(inputs["edge_index"], np.int64)
    loops = np.arange(N, dtype=np.int64)
    src = np.concatenate([ei[0], loops])
    dst = np.concatenate([ei[1], loops])
    deg = np.bincount(dst, minlength=N).astype(np.float32)
    dinv = (1.0 / np.sqrt(deg)).astype(np.float32)
    xs_pad = np.zeros((N, HID), np.float32)
    xs_pad[:, :IN] = x
    xs_pad = xs_pad.astype(NP_TAB)

    core = dst // NPC
    rem = dst - core * NPC
    tidx = rem >> 7
    loc = (rem & 127).astype(np.float32)
    half = (src >= SPLIT).astype(np.int64)
    nrm = dinv[src] * dinv[dst]

    order = np.lexsort((half, tidx, core))
    src_s = src[order]
    core_s = core[order]
    tidx_s = tidx[order]
    half_s = half[order]
    loc_s = loc[order]
    nrm_s = nrm[order]

    gk = (core_s * NT + tidx_s) * 2 + half_s
    cnt = np.bincount(gk, minlength=NCORES * NT * 2).reshape(NCORES, NT, 2)
    S2 = np.ceil(cnt.max(axis=0) / P).astype(np.int64)  # [NT, 2]
    T_L = int(S2[:, 0].sum())
    TT = T_L + int(S2[:, 1].sum())
    offL = np.zeros(NT, np.int64)
    offL[1:] = np.cumsum(S2[:, 0])[:-1]
    offH = np.zeros(NT, np.int64)
    offH[1:] = np.cumsum(S2[:, 1])[:-1]

    starts = np.zeros(NCORES * NT * 2, np.int64)
    starts[1:] = np.cumsum(cnt.reshape(-1))[:-1]
    pos = np.arange(len(src_s)) - starts[gk]
    subl = pos >> 7
    lane = pos & 127
    gsub = np.where(half_s == 0, offL[tidx_s], T_L + offH[tidx_s]) + subl

    gidx = np.zeros((NCORES, P, TT), np.int16)
    dloc = np.full((NCORES, P, TT), 1000.0, np.float32)
    nrmv = np.zeros((NCORES, P, TT), np.float32)
    gidx[core_s, lane, gsub] = (src_s - half_s * SPLIT).astype(np.int16)
    dloc[core_s, lane, gsub] = loc_s
    nrmv[core_s, lane, gsub] = nrm_s

    # dma_gather idx layout: global position i=(gsub*128+lane) -> [i%16, i//16],
    # replicated across the 8 groups of 16 partitions.
    idx16 = gidx.transpose(0, 2, 1).reshape(NCORES, TT * 8, 16).transpose(0, 2, 1)
    idx_tile = np.tile(idx16, (1, 8, 1))  # [NCORES, 128, TT*8]

    com = {
        "xs": np.ascontiguousarray(xs_pad),
        "W1": np.asarray(inputs["W1"], np.float32),
        "W2": np.asarray(inputs["W2"], np.float32),
        "W3": np.asarray(inputs["W3"], np.float32),
        "fcW": np.asarray(inputs["fcW"], np.float32).reshape(HID, 1),
        "g1": np.asarray(inputs["g1"], np.float32).reshape(HID, 1),
        "g2": np.asarray(inputs["g2"], np.float32).reshape(HID, 1),
        "bt1": np.asarray(inputs["bt1"], np.float32).reshape(HID, 1),
        "bt2": np.asarray(inputs["bt2"], np.float32).reshape(HID, 1),
        "b3": np.asarray(inputs["b3"], np.float32).reshape(HID, 1),
    }
    in_maps = []
    for c in range(NCORES):
        m = dict(com)
        m["gidx"] = np.ascontiguousarray(idx_tile[c])
        m["dloc"] = np.ascontiguousarray(dloc[c])
        m["nrm"] = np.ascontiguousarray(nrmv[c])
        in_maps.append(m)
    return in_maps, tuple(tuple(int(v) for v in row) for row in S2)


def _get_nc(S2):
    key = (S2, REPS, GMAX, LAYERS)
    if key not in _NC_CACHE:
        _NC_CACHE[key] = _build(S2)
    return _NC_CACHE[key]


class _Exec:
    """jit-once / device_put-once executor mirroring bass2jax.run_bass_via_pjrt."""

    def __init__(self, nc, in_maps):
        import jax
        from jax.sharding import Mesh, PartitionSpec
        from jax.experimental.shard_map import shard_map
        from concourse import bass2jax
        bass2jax.install_neuronx_cc_hook()
        n_cores = NCORES
        part_name = nc.partition_id_tensor.name if nc.partition_id_tensor else None
        in_names, out_names, out_avals, zero_outs = [], [], [], []
        for alloc in nc.m.functions[0].allocations:
            if not isinstance(alloc, mybir.MemoryLocationSet):
                continue
            name = alloc.memorylocations[0].name
            if alloc.kind == "ExternalInput":
                if name != part_name:
                    in_names.append(name)
            elif alloc.kind == "ExternalOutput":
                out_names.append(name)
                shape = tuple(alloc.tensor_shape)
                dtype = mybir.dt.np(alloc.dtype)
                out_avals.append(jax.core.ShapedArray(shape, dtype))
                zero_outs.append(np.zeros(shape, dtype))
        n_params = len(in_names)
        all_names = in_names + out_names
        if part_name is not None:
            all_names = all_names + [part_name]
        self.out_names, self.out_avals, self.n_cores = out_names, out_avals, n_cores

        def _body(*args):
            operands = list(args)
            if part_name is not None:
                operands.append(bass2jax.partition_id_tensor())
            outs = bass2jax._bass_exec_p.bind(
                *operands,
                out_avals=tuple(out_avals),
                in_names=tuple(all_names),
                out_names=tuple(out_names),
                lowering_input_output_aliases=(),
                sim_require_finite=True,
                sim_require_nnan=True,
                nc=nc,
            )
            return tuple(outs)

        devices = jax.devices()[:n_cores]
        mesh = Mesh(np.asarray(devices), ("core",))
        in_specs = (PartitionSpec("core"),) * (n_params + len(out_names))
        out_specs = (PartitionSpec("core"),) * len(out_names)
        self.fn = jax.jit(
            shard_map(_body, mesh=mesh, in_specs=in_specs, out_specs=out_specs,
                      check_rep=False),
            keep_unused=True,
        )
        concat_in = [
            np.concatenate([np.asarray(in_maps[c][k]) for c in range(n_cores)], axis=0)
            for k in in_names
        ]
        concat_zeros = [
            np.zeros((n_cores * z.shape[0], *z.shape[1:]), z.dtype) for z in zero_outs
        ]
        sh = jax.sharding.NamedSharding(mesh, PartitionSpec("core"))
        self.dev_in = [jax.device_put(a, sh) for a in concat_in] + \
                      [jax.device_put(a, sh) for a in concat_zeros]
        for a in self.dev_in:
            a.block_until_ready()

    def run(self):
        outs = self.fn(*self.dev_in)
        for o in outs:
            o.block_until_ready()
        return outs

    def results(self):
        outs = self.run()
        res = [dict() for _ in range(self.n_cores)]
        for i, name in enumerate(self.out_names):
            arr = np.asarray(outs[i]).reshape(self.n_cores, *self.out_avals[i].shape)
            for c in range(self.n_cores):
                res[c][name] = arr[c]
        return res


_EXEC_CACHE = {}


def _get_exec(in_maps, S2):
    key = (S2, REPS, GMAX, LAYERS)
    if key not in _EXEC_CACHE:
        _EXEC_CACHE[key] = _Exec(_get_nc(S2), in_maps)
    return _EXEC_CACHE[key]


def _run(in_maps, S2):
    nc = _get_nc(S2)
    r = bass_utils.run_bass_kernel_spmd(nc, in_maps, core_ids=list(range(NCORES)), trace=False)
    return r


def kernel(**inputs):
    in_maps, S2 = _prep(inputs)
    r = _run(in_maps, S2)
    out = np.concatenate([r.results[c]["outv"].reshape(-1) for c in range(NCORES)])
    fcb = np.asarray(inputs["fcb"], np.float32).reshape(-1)
    out = (out + fcb[0]).astype(np.float32)[:, None]
    # numerically stable sigmoid in fp32
    sig = np.empty_like(out)
    pos = out >= 0
    sig[pos] = 1.0 / (1.0 + np.exp(-out[pos], dtype=np.float32))
    ex = np.exp(out[~pos], dtype=np.float32)
    sig[~pos] = ex / (1.0 + ex)
    return out, sig


# revision 7
# speedup vs baseline: 1.4886x; 1.4886x over previous
"""BindingPocketGNN (3-layer GCN, N=50000, E=800000) on 8 Trainium2 NeuronCores.

Distribution: nodes sharded into 8 contiguous ranges (6250/core). Each core owns the
scatter/aggregation for its destination-node range; edges are routed (host-side) to the
core owning their destination. Source features come from a replicated node-major table
(input x for layer 1; AllGather-replicated activations for layers 2/3).

v2: the per-edge gather uses a few big gpsimd dma_gather (InstDMAGatherAnt) calls per
layer (0.34ns/descriptor batched SWDGE) instead of ~900 small indirect_dma_start ops
(~1us fixed each, which made the baseline SWDGE/gpsimd-bound at 87% occupancy).
dma_gather takes int16 indices, so the 50000-row table is addressed as two halves
(src < 32768 via table[0:32768], src >= 32768 via table[32768:]); edges are grouped
host-side by (dst tile, half) into 128-edge subtiles, padded with idx=0 slots that a
one-hot mask zeroes out.

Per layer, on each core (feat-major formulation so BN/bias are per-partition):
    msg[e, f]     = table[src_e, f]                      (chunked dma_gather, bf16)
    mask[e, d]    = (iota[d]==dloc[e]) * norm_e          (DVE tensor_scalar, one op)
    z^T[f, d]    += msg^T @ mask                         (TensorE accumulate over subtiles)
    y^T[f_out, d] = W^T @ z^T                            (TensorE)
    stats         = AllReduce(sum/sumsq of y)            (1KB collective; BN layers)
    act^T         = Relu(A*y^T + B)                      (ScalarE, per-partition A/B)
    h             = act^T transposed to node-major       (TensorE transpose)
    table_{l+1}   = AllGather(h)                         (collective; layers 1,2)
Layer 3 feeds a [128->1] FC matmul; +fcb and sigmoid applied on host.

Tables are prescaled by dinv[src] (host for x, ACT-scale in the epilogue for h);
dinv[dst] is applied per dst tile via a broadcast row table (dinv_bc). Masks are pure
0/1 bf16 built with tensor_tensor is_equal (per-partition-scalar AP operands on DVE
cost ~1us/op in scalar-fetch mode; the broadcast tensor_tensor form does not).
"""
import sys
if "/opt/trn_rl_repo" not in sys.path:
    sys.path.insert(0, "/opt/trn_rl_repo")

import os
import numpy as np
import ml_dtypes

import concourse.bass as bass
import concourse.bacc as bacc
import concourse.mybir as mybir
import concourse.tile as tile
from concourse import bass_utils
from concourse.masks import make_identity

N = 50000
E = 800000
IN, HID = 64, 128
BN_EPS = 1e-5
NCORES = 8
NPC = N // NCORES          # 6250 nodes per core
P = 128
NT = (NPC + P - 1) // P    # 49 dst tiles per core
LAST_D = NPC - (NT - 1) * P  # 106
SPLIT = 32768              # int16 index range boundary for dma_gather

F32 = mybir.dt.float32
I16 = mybir.dt.int16
BF16 = mybir.dt.bfloat16
DT_TAB = BF16
NP_TAB = np.dtype(ml_dtypes.bfloat16)

REPS = int(os.environ.get("GCN_REPS", "1"))
GMAX = int(os.environ.get("GCN_GMAX", "72"))   # max subtiles per dma_gather chunk
LAYERS = int(os.environ.get("GCN_LAYERS", "3"))  # debug: run only first k layers

Alu = mybir.AluOpType
Act = mybir.ActivationFunctionType

_NC_CACHE = {}


def _chunk_plan(S2):
    """Greedy-pack tiles' subtile groups into gather chunks of <= GMAX subtiles.
    Returns (chunks, per_tile): chunks[h] = list of (sub_lo_in_half, n_sub);
    per_tile[h][t] = (chunk_id, col0) locating tile t's first subtile of half h."""
    chunks, per_tile = [], []
    for h in (0, 1):
        ch, pt = [], []
        lo, n = 0, 0
        for t in range(NT):
            s = S2[t][h]
            if n + s > GMAX and n > 0:
                ch.append((lo, n))
                lo += n
                n = 0
            pt.append((len(ch), n))
            n += s
        if n:
            ch.append((lo, n))
        chunks.append(ch)
        per_tile.append(pt)
    return chunks, per_tile


def _build(S2):
    """Build+schedule the SPMD program. S2 = tuple of (S_low, S_high) per dst tile;
    identical for all 8 cores."""
    S2 = [tuple(s) for s in S2]
    T_L = sum(s[0] for s in S2)
    T_H = sum(s[1] for s in S2)
    TT = T_L + T_H
    chunks, per_tile = _chunk_plan(S2)
    # global subtile index (dloc/norm column): low half at sub_lo, high at T_L + sub_lo
    half_base = (0, T_L)

    nc = bacc.Bacc("TRN2", target_bir_lowering=False, debug=False, num_devices=NCORES,
                   num_swdge_queues=4)

    # ---- I/O ----
    xs = nc.dram_tensor("xs", [N, HID], DT_TAB, kind="ExternalInput")
    gidx_d = nc.dram_tensor("gidx", [P, TT * 8], I16, kind="ExternalInput")
    dloc_d = nc.dram_tensor("dloc", [P, TT], DT_TAB, kind="ExternalInput")
    dinv_d = nc.dram_tensor("dinv_sl", [P, NT], F32, kind="ExternalInput")
    W_d = [
        nc.dram_tensor("W1", [IN, HID], F32, kind="ExternalInput"),
        nc.dram_tensor("W2", [HID, HID], F32, kind="ExternalInput"),
        nc.dram_tensor("W3", [HID, HID], F32, kind="ExternalInput"),
    ]
    fcW_d = nc.dram_tensor("fcW", [HID, 1], F32, kind="ExternalInput")
    g_d = [nc.dram_tensor("g1", [HID, 1], F32, kind="ExternalInput"),
           nc.dram_tensor("g2", [HID, 1], F32, kind="ExternalInput")]
    bt_d = [nc.dram_tensor("bt1", [HID, 1], F32, kind="ExternalInput"),
            nc.dram_tensor("bt2", [HID, 1], F32, kind="ExternalInput")]
    b3_d = nc.dram_tensor("b3", [HID, 1], F32, kind="ExternalInput")
    outv = nc.dram_tensor("outv", [1, NPC], F32, kind="ExternalOutput")

    with tile.TileContext(nc) as tc:
        with (
            tc.tile_pool(name="meta", bufs=1) as meta,
            tc.tile_pool(name="msgLp", bufs=2) as msgLp,
            tc.tile_pool(name="msgHp", bufs=2) as msgHp,
            tc.tile_pool(name="maskp", bufs=8) as maskp,
            tc.tile_pool(name="zsp", bufs=3) as zsp,
            tc.tile_pool(name="actp", bufs=3) as actp,
            tc.tile_pool(name="hp", bufs=3) as hp,
            tc.tile_pool(name="sqp", bufs=2) as sqp,
            tc.tile_pool(name="zps_p", bufs=2, space="PSUM") as zps_p,
            tc.tile_pool(name="yps_p", bufs=2, space="PSUM") as yps_p,
            tc.tile_pool(name="trps_p", bufs=2, space="PSUM") as trps_p,
            tc.tile_pool(name="fcps_p", bufs=1, space="PSUM") as fcps_p,
            tc.tile_pool(name="dram", bufs=1, space="DRAM") as dram,
        ):
            # ---- resident metadata ----
            gidx_sb = meta.tile([P, TT * 8], I16)
            nc.sync.dma_start(gidx_sb[:], gidx_d[:])
            dloc_sb = meta.tile([P, TT], DT_TAB)
            nc.sync.dma_start(dloc_sb[:], dloc_d[:])
            dinv_sl = meta.tile([P, NT], F32)
            nc.sync.dma_start(dinv_sl[:], dinv_d[:])
            W_sb = []
            for l in range(3):
                fi = IN if l == 0 else HID
                w = meta.tile([fi, HID], F32, name=f"W{l}_sb")
                nc.sync.dma_start(w[:], W_d[l][:])
                W_sb.append(w)
            fcW_sb = meta.tile([HID, 1], F32)
            nc.sync.dma_start(fcW_sb[:], fcW_d[:])
            g_sb, bt_sb = [], []
            for l in range(2):
                gg = meta.tile([HID, 1], F32, name=f"g{l}_sb")
                nc.sync.dma_start(gg[:], g_d[l][:])
                g_sb.append(gg)
                bb = meta.tile([HID, 1], F32, name=f"bt{l}_sb")
                nc.sync.dma_start(bb[:], bt_d[l][:])
                bt_sb.append(bb)
            b3_sb = meta.tile([HID, 1], F32)
            nc.sync.dma_start(b3_sb[:], b3_d[:])
            eps_sb = meta.tile([P, 1], F32)
            nc.vector.memset(eps_sb[:], BN_EPS)

            ident = meta.tile([P, P], F32)
            make_identity(nc, ident[:])
            iota_i = meta.tile([P, P], mybir.dt.int32)
            nc.gpsimd.iota(iota_i[:], pattern=[[1, P]], base=0, channel_multiplier=0)
            iota_t = meta.tile([P, P], DT_TAB)
            nc.vector.tensor_copy(iota_t[:], iota_i[:])

            # dinv broadcast rows: dinv_bc[:, t*128+j] = dinv of node t*128+j (all partitions)
            dinv_bc = meta.tile([P, NT * P], F32)
            for t in range(NT):
                tr = trps_p.tile([P, P], F32, tag="tr")
                nc.tensor.transpose(tr[:], dinv_sl[:, t:t + 1].to_broadcast([P, P]), ident[:])
                nc.vector.tensor_copy(dinv_bc[:, t * P:(t + 1) * P], tr[:])

            ystore = meta.tile([P, NT * P], F32)
            sums = meta.tile([P, NT], F32)
            sumsq = meta.tile([P, NT], F32)
            out_store = meta.tile([1, NPC], F32)

            # internal DRAM for collectives (fresh per rep: Shared tensors allow one writer)
            def mk_coll(rep):
                tab_in = [dram.tile([NPC, HID], DT_TAB, name=f"tab{l}_in_r{rep}") for l in (1, 2)]
                tab_out = [dram.tile([N, HID], DT_TAB, name=f"tab{l}_out_r{rep}",
                                     addr_space="Shared") for l in (1, 2)]
                st_in = [dram.tile([P, 2], F32, name=f"st{l}_in_r{rep}") for l in (0, 1)]
                st_out = [dram.tile([P, 2], F32, name=f"st{l}_out_r{rep}", addr_space="Shared")
                          for l in (0, 1)]
                return tab_in, tab_out, st_in, st_out

            for _rep in range(REPS):
              tab_in, tab_out, st_in, st_out = mk_coll(_rep)
              for l in range(LAYERS):
                 f_in = IN if l == 0 else HID
                 table = xs if l == 0 else tab_out[l - 1]

                 # ---- chunked gathers (issued lazily, consumed tile-major) ----
                 chunk_tiles = {}
                 qrot = [l % 4]

                 def get_chunk(h, cid, l=l, table=table, chunk_tiles=chunk_tiles, qrot=qrot):
                     key = (h, cid)
                     if key not in chunk_tiles:
                         sub_lo, nsub = chunks[h][cid]
                         pool = msgLp if h == 0 else msgHp
                         buf = pool.tile([P, GMAX * HID], DT_TAB, tag=f"msg{h}")
                         src_ap = table[0:SPLIT, :] if h == 0 else table[SPLIT:N, :]
                         g0 = half_base[h] + sub_lo
                         nidx = nsub * P
                         nc.gpsimd.dma_gather(
                             buf[:, :nsub * HID].rearrange("p (g e) -> p g e", e=HID),
                             src_ap,
                             gidx_sb[:, g0 * 8:(g0 + nsub) * 8],
                             nidx, nidx, HID, queue_num=qrot[0], single_packet=False,
                         )
                         qrot[0] = (qrot[0] + 1) % 4
                         chunk_tiles[key] = buf
                     return chunk_tiles[key]

                 # ---- aggregation + weight matmul ----
                 for t in range(NT):
                     d_hi = LAST_D if t == NT - 1 else P
                     segs = []
                     for h in (0, 1):
                         sth = S2[t][h]
                         if sth:
                             segs.append((h, per_tile[h][t][0], per_tile[h][t][1], sth))
                     ntot = sum(s[3] for s in segs)
                     zps = zps_p.tile([P, P], F32, tag="zps")
                     k = 0
                     for (h, cid, col0, sth) in segs:
                         buf = get_chunk(h, cid)
                         g0 = half_base[h] + chunks[h][cid][0] + col0
                         for s in range(sth):
                             g = g0 + s
                             mask = maskp.tile([P, P], DT_TAB, tag="mask")
                             nc.vector.tensor_tensor(
                                 out=mask[:], in0=iota_t[:],
                                 in1=dloc_sb[:, g:g + 1].to_broadcast([P, P]),
                                 op=Alu.is_equal,
                             )
                             nc.tensor.matmul(
                                 zps[:f_in, :],
                                 lhsT=buf[:, (col0 + s) * HID:(col0 + s) * HID + f_in],
                                 rhs=mask[:],
                                 start=(k == 0), stop=(k == ntot - 1),
                             )
                             k += 1
                     zs = zsp.tile([P, P], F32, tag="zs")
                     nc.vector.tensor_tensor(
                         out=zs[:f_in, :], in0=zps[:f_in, :],
                         in1=dinv_bc[:f_in, t * P:(t + 1) * P], op=Alu.mult,
                     )
                     yps = yps_p.tile([P, P], F32, tag="yps")
                     nc.tensor.matmul(yps[:], lhsT=W_sb[l][:], rhs=zs[:f_in, :],
                                      start=True, stop=True)
                     if l < 2:
                         nc.scalar.activation(
                             out=ystore[:, t * P:t * P + d_hi], in_=yps[:, :d_hi],
                             func=Act.Copy, accum_out=sums[:, t:t + 1],
                         )
                         sq = sqp.tile([P, P], F32, tag="sq")
                         nc.scalar.activation(
                             out=sq[:, :d_hi], in_=yps[:, :d_hi],
                             func=Act.Square, accum_out=sumsq[:, t:t + 1],
                         )
                     else:
                         act3 = actp.tile([P, P], F32, tag="act")
                         nc.scalar.activation(out=act3[:, :d_hi], in_=yps[:, :d_hi],
                                              func=Act.Relu, bias=b3_sb[:], scale=1.0)
                         fcp = fcps_p.tile([1, P], F32, tag="fcp")
                         nc.tensor.matmul(fcp[:1, :d_hi], lhsT=fcW_sb[:], rhs=act3[:, :d_hi],
                                          start=True, stop=True)
                         nc.vector.tensor_copy(out_store[:1, t * P:t * P + d_hi], fcp[:1, :d_hi])

                 if l == LAYERS - 1 and l < 2:
                     nc.vector.tensor_copy(out_store[:1, :], ystore[:1, :NPC])
                     break
                 if l < 2:
                     # ---- BN stats allreduce + coefficients ----
                     stats = meta.tile([P, 2], F32, name=f"stats{l}_r{_rep}")
                     nc.vector.tensor_reduce(stats[:, 0:1], sums[:], axis=mybir.AxisListType.X, op=Alu.add)
                     nc.vector.tensor_reduce(stats[:, 1:2], sumsq[:], axis=mybir.AxisListType.X, op=Alu.add)
                     nc.sync.dma_start(st_in[l][:], stats[:])
                     nc.gpsimd.collective_compute(
                         "AllReduce", Alu.add, replica_groups=[list(range(NCORES))],
                         ins=[st_in[l][:]], outs=[st_out[l][:]],
                     )
                     tot = meta.tile([P, 2], F32, name=f"tot{l}_r{_rep}")
                     nc.sync.dma_start(tot[:], st_out[l][:])
                     cf = meta.tile([P, 6], F32, name=f"cf{l}_r{_rep}")  # mean ex2 var std A B
                     nc.vector.tensor_scalar_mul(cf[:, 0:1], tot[:, 0:1], 1.0 / N)
                     nc.vector.tensor_scalar_mul(cf[:, 1:2], tot[:, 1:2], 1.0 / N)
                     nc.vector.tensor_tensor(out=cf[:, 2:3], in0=cf[:, 0:1], in1=cf[:, 0:1], op=Alu.mult)
                     nc.vector.tensor_tensor(out=cf[:, 2:3], in0=cf[:, 1:2], in1=cf[:, 2:3], op=Alu.subtract)
                     nc.scalar.activation(out=cf[:, 3:4], in_=cf[:, 2:3], func=Act.Sqrt, bias=eps_sb[:], scale=1.0)
                     nc.vector.reciprocal(cf[:, 4:5], cf[:, 3:4])
                     A = meta.tile([P, 1], F32, name=f"A{l}_r{_rep}")
                     B = meta.tile([P, 1], F32, name=f"B{l}_r{_rep}")
                     nc.vector.tensor_tensor(out=A[:], in0=g_sb[l][:], in1=cf[:, 4:5], op=Alu.mult)
                     nc.vector.tensor_tensor(out=cf[:, 5:6], in0=cf[:, 0:1], in1=A[:], op=Alu.mult)
                     nc.vector.tensor_tensor(out=B[:], in0=bt_sb[l][:], in1=cf[:, 5:6], op=Alu.subtract)

                     # ---- epilogue: act, transpose to node-major, store table slice ----
                     for t in range(NT):
                         d_hi = LAST_D if t == NT - 1 else P
                         act = actp.tile([P, P], F32, tag="act")
                         nc.scalar.activation(out=act[:, :d_hi], in_=ystore[:, t * P:t * P + d_hi],
                                              func=Act.Relu, bias=B[:], scale=A[:])
                         tr = trps_p.tile([P, P], F32, tag="tr")
                         nc.tensor.transpose(tr[:d_hi, :], act[:, :d_hi], ident[:])
                         h = hp.tile([P, HID], DT_TAB, tag="h")
                         nc.scalar.activation(out=h[:d_hi, :], in_=tr[:d_hi, :],
                                              func=Act.Copy, scale=dinv_sl[:d_hi, t:t + 1])
                         nc.sync.dma_start(tab_in[l][t * P:t * P + d_hi, :], h[:d_hi, :])
                     nc.gpsimd.collective_compute(
                         "AllGather", Alu.bypass, replica_groups=[list(range(NCORES))],
                         ins=[tab_in[l][:]], outs=[tab_out[l][:]],
                     )

            nc.sync.dma_start(outv[:], out_store[:])

    nc.compile()
    return nc


def _prep(inputs):
    x = np.asarray(inputs["x"], np.float32)
    ei = np.asarray(inputs["edge_index"], np.int64)
    loops = np.arange(N, dtype=np.int64)
    src = np.concatenate([ei[0], loops])
    dst = np.concatenate([ei[1], loops])
    deg = np.bincount(dst, minlength=N).astype(np.float32)
    dinv = (1.0 / np.sqrt(deg)).astype(np.float32)
    xs_pad = np.zeros((N, HID), np.float32)
    xs_pad[:, :IN] = x * dinv[:, None]
    xs_pad = xs_pad.astype(NP_TAB)

    core = dst // NPC
    rem = dst - core * NPC
    tidx = rem >> 7
    loc = (rem & 127).astype(np.float32)
    half = (src >= SPLIT).astype(np.int64)

    order = np.lexsort((half, tidx, core))
    src_s = src[order]
    core_s = core[order]
    tidx_s = tidx[order]
    half_s = half[order]
    loc_s = loc[order]

    gk = (core_s * NT + tidx_s) * 2 + half_s
    cnt = np.bincount(gk, minlength=NCORES * NT * 2).reshape(NCORES, NT, 2)
    S2 = np.ceil(cnt.max(axis=0) / P).astype(np.int64)  # [NT, 2]
    T_L = int(S2[:, 0].sum())
    TT = T_L + int(S2[:, 1].sum())
    offL = np.zeros(NT, np.int64)
    offL[1:] = np.cumsum(S2[:, 0])[:-1]
    offH = np.zeros(NT, np.int64)
    offH[1:] = np.cumsum(S2[:, 1])[:-1]

    starts = np.zeros(NCORES * NT * 2, np.int64)
    starts[1:] = np.cumsum(cnt.reshape(-1))[:-1]
    pos = np.arange(len(src_s)) - starts[gk]
    subl = pos >> 7
    lane = pos & 127
    gsub = np.where(half_s == 0, offL[tidx_s], T_L + offH[tidx_s]) + subl

    gidx = np.zeros((NCORES, P, TT), np.int16)
    dloc = np.full((NCORES, P, TT), 1000.0, np.float32)
    gidx[core_s, lane, gsub] = (src_s - half_s * SPLIT).astype(np.int16)
    dloc[core_s, lane, gsub] = loc_s

    dinv_pad = np.zeros((NCORES, NT * P), np.float32)
    dinv_pad[:, :NPC] = dinv.reshape(NCORES, NPC)
    dinv_sl = dinv_pad.reshape(NCORES, NT, P).transpose(0, 2, 1).copy()  # [c, P, NT]

    # dma_gather idx layout: global position i=(gsub*128+lane) -> [i%16, i//16],
    # replicated across the 8 groups of 16 partitions.
    idx16 = gidx.transpose(0, 2, 1).reshape(NCORES, TT * 8, 16).transpose(0, 2, 1)
    idx_tile = np.tile(idx16, (1, 8, 1))  # [NCORES, 128, TT*8]

    com = {
        "xs": np.ascontiguousarray(xs_pad),
        "W1": np.asarray(inputs["W1"], np.float32),
        "W2": np.asarray(inputs["W2"], np.float32),
        "W3": np.asarray(inputs["W3"], np.float32),
        "fcW": np.asarray(inputs["fcW"], np.float32).reshape(HID, 1),
        "g1": np.asarray(inputs["g1"], np.float32).reshape(HID, 1),
        "g2": np.asarray(inputs["g2"], np.float32).reshape(HID, 1),
        "bt1": np.asarray(inputs["bt1"], np.float32).reshape(HID, 1),
        "bt2": np.asarray(inputs["bt2"], np.float32).reshape(HID, 1),
        "b3": np.asarray(inputs["b3"], np.float32).reshape(HID, 1),
    }
    in_maps = []
    for c in range(NCORES):
        m = dict(com)
        m["gidx"] = np.ascontiguousarray(idx_tile[c])
        m["dloc"] = np.ascontiguousarray(dloc[c].astype(NP_TAB))
        m["dinv_sl"] = np.ascontiguousarray(dinv_sl[c])
        in_maps.append(m)
    return in_maps, tuple(tuple(int(v) for v in row) for row in S2)


def _get_nc(S2):
    key = (S2, REPS, GMAX, LAYERS)
    if key not in _NC_CACHE:
        _NC_CACHE[key] = _build(S2)
    return _NC_CACHE[key]


class _Exec:
    """jit-once / device_put-once executor mirroring bass2jax.run_bass_via_pjrt."""

    def __init__(self, nc, in_maps):
        import jax
        from jax.sharding import Mesh, PartitionSpec
        from jax.experimental.shard_map import shard_map
        from concourse import bass2jax
        bass2jax.install_neuronx_cc_hook()
        n_cores = NCORES
        part_name = nc.partition_id_tensor.name if nc.partition_id_tensor else None
        in_names, out_names, out_avals, zero_outs = [], [], [], []
        for alloc in nc.m.functions[0].allocations:
            if not isinstance(alloc, mybir.MemoryLocationSet):
                continue
            name = alloc.memorylocations[0].name
            if alloc.kind == "ExternalInput":
                if name != part_name:
                    in_names.append(name)
            elif alloc.kind == "ExternalOutput":
                out_names.append(name)
                shape = tuple(alloc.tensor_shape)
                dtype = mybir.dt.np(alloc.dtype)
                out_avals.append(jax.core.ShapedArray(shape, dtype))
                zero_outs.append(np.zeros(shape, dtype))
        n_params = len(in_names)
        all_names = in_names + out_names
        if part_name is not None:
            all_names = all_names + [part_name]
        self.out_names, self.out_avals, self.n_cores = out_names, out_avals, n_cores

        def _body(*args):
            operands = list(args)
            if part_name is not None:
                operands.append(bass2jax.partition_id_tensor())
            outs = bass2jax._bass_exec_p.bind(
                *operands,
                out_avals=tuple(out_avals),
                in_names=tuple(all_names),
                out_names=tuple(out_names),
                lowering_input_output_aliases=(),
                sim_require_finite=True,
                sim_require_nnan=True,
                nc=nc,
            )
            return tuple(outs)

        devices = jax.devices()[:n_cores]
        mesh = Mesh(np.asarray(devices), ("core",))
        in_specs = (PartitionSpec("core"),) * (n_params + len(out_names))
        out_specs = (PartitionSpec("core"),) * len(out_names)
        self.fn = jax.jit(
            shard_map(_body, mesh=mesh, in_specs=in_specs, out_specs=out_specs,
                      check_rep=False),
            keep_unused=True,
        )
        concat_in = [
            np.concatenate([np.asarray(in_maps[c][k]) for c in range(n_cores)], axis=0)
            for k in in_names
        ]
        concat_zeros = [
            np.zeros((n_cores * z.shape[0], *z.shape[1:]), z.dtype) for z in zero_outs
        ]
        sh = jax.sharding.NamedSharding(mesh, PartitionSpec("core"))
        self.dev_in = [jax.device_put(a, sh) for a in concat_in] + \
                      [jax.device_put(a, sh) for a in concat_zeros]
        for a in self.dev_in:
            a.block_until_ready()

    def run(self):
        outs = self.fn(*self.dev_in)
        for o in outs:
            o.block_until_ready()
        return outs

    def results(self):
        outs = self.run()
        res = [dict() for _ in range(self.n_cores)]
        for i, name in enumerate(self.out_names):
            arr = np.asarray(outs[i]).reshape(self.n_cores, *self.out_avals[i].shape)
            for c in range(self.n_cores):
                res[c][name] = arr[c]
        return res


_EXEC_CACHE = {}


def _get_exec(in_maps, S2):
    key = (S2, REPS, GMAX, LAYERS)
    if key not in _EXEC_CACHE:
        _EXEC_CACHE[key] = _Exec(_get_nc(S2), in_maps)
    return _EXEC_CACHE[key]


def _run(in_maps, S2):
    nc = _get_nc(S2)
    r = bass_utils.run_bass_kernel_spmd(nc, in_maps, core_ids=list(range(NCORES)), trace=False)
    return r


def kernel(**inputs):
    in_maps, S2 = _prep(inputs)
    r = _run(in_maps, S2)
    out = np.concatenate([r.results[c]["outv"].reshape(-1) for c in range(NCORES)])
    fcb = np.asarray(inputs["fcb"], np.float32).reshape(-1)
    out = (out + fcb[0]).astype(np.float32)[:, None]
    # numerically stable sigmoid in fp32
    sig = np.empty_like(out)
    pos = out >= 0
    sig[pos] = 1.0 / (1.0 + np.exp(-out[pos], dtype=np.float32))
    ex = np.exp(out[~pos], dtype=np.float32)
    sig[~pos] = ex / (1.0 + ex)
    return out, sig


# revision 10
# speedup vs baseline: 2.3324x; 1.5668x over previous
"""BindingPocketGNN (3-layer GCN, N=50000, E=800000) on 8 Trainium2 NeuronCores.

Distribution: nodes sharded into 8 contiguous ranges (6250/core). Each core owns the
scatter/aggregation for its destination-node range; edges are routed (host-side) to the
core owning their destination. Source features come from a replicated node-major table
(input x for layer 1; AllGather-replicated activations for layers 2/3).

v2: the per-edge gather uses a few big gpsimd dma_gather (InstDMAGatherAnt) calls per
layer (0.34ns/descriptor batched SWDGE) instead of ~900 small indirect_dma_start ops
(~1us fixed each, which made the baseline SWDGE/gpsimd-bound at 87% occupancy).
dma_gather takes int16 indices, so the 50000-row table is addressed as two halves
(src < 32768 via table[0:32768], src >= 32768 via table[32768:]); edges are grouped
host-side by (dst tile, half) into 128-edge subtiles, padded with idx=0 slots that a
one-hot mask zeroes out.

Per layer, on each core (feat-major formulation so BN/bias are per-partition):
    msg[e, f]     = table[src_e, f]                      (chunked dma_gather, bf16)
    mask[e, d]    = (iota[d]==dloc[e]) * norm_e          (DVE tensor_scalar, one op)
    z^T[f, d]    += msg^T @ mask                         (TensorE accumulate over subtiles)
    y^T[f_out, d] = W^T @ z^T                            (TensorE)
    stats         = AllReduce(sum/sumsq of y)            (1KB collective; BN layers)
    act^T         = Relu(A*y^T + B)                      (ScalarE, per-partition A/B)
    h             = act^T transposed to node-major       (TensorE transpose)
    table_{l+1}   = AllGather(h)                         (collective; layers 1,2)
Layer 3 feeds a [128->1] FC matmul; +fcb and sigmoid applied on host.

Tables are prescaled by dinv[src] (host for x, ACT-scale in the epilogue for h);
dinv[dst] is applied per dst tile via a broadcast row table (dinv_bc). Masks are pure
0/1 bf16 built with tensor_tensor is_equal (per-partition-scalar AP operands on DVE
cost ~1us/op in scalar-fetch mode; the broadcast tensor_tensor form does not).
"""
import sys
if "/opt/trn_rl_repo" not in sys.path:
    sys.path.insert(0, "/opt/trn_rl_repo")

import os
import numpy as np
import ml_dtypes

import concourse.bass as bass
import concourse.bacc as bacc
import concourse.mybir as mybir
import concourse.tile as tile
from concourse import bass_utils
from concourse.masks import make_identity

N = 50000
E = 800000
IN, HID = 64, 128
BN_EPS = 1e-5
NCORES = 8
NPC = N // NCORES          # 6250 nodes per core
P = 128
NT = (NPC + P - 1) // P    # 49 dst tiles per core
LAST_D = NPC - (NT - 1) * P  # 106
SPLIT = 32768              # int16 index range boundary for dma_gather

F32 = mybir.dt.float32
I16 = mybir.dt.int16
BF16 = mybir.dt.bfloat16
DT_TAB = BF16
NP_TAB = np.dtype(ml_dtypes.bfloat16)

REPS = int(os.environ.get("GCN_REPS", "1"))
GMAX = int(os.environ.get("GCN_GMAX", "24"))   # max subtiles per dma_gather chunk
LAYERS = int(os.environ.get("GCN_LAYERS", "3"))  # debug: run only first k layers
MB = 8                                           # mask-build batch (subtiles per DVE op)

Alu = mybir.AluOpType
Act = mybir.ActivationFunctionType

_NC_CACHE = {}


def _chunk_plan(S2):
    """Greedy-pack tiles' subtile groups into gather chunks of <= GMAX subtiles.
    Returns (chunks, per_tile): chunks[h] = list of (sub_lo_in_half, n_sub);
    per_tile[h][t] = (chunk_id, col0) locating tile t's first subtile of half h."""
    chunks, per_tile = [], []
    for h in (0, 1):
        ch, pt = [], []
        lo, n = 0, 0
        for t in range(NT):
            s = S2[t][h]
            if n + s > GMAX and n > 0:
                ch.append((lo, n))
                lo += n
                n = 0
            pt.append((len(ch), n))
            n += s
        if n:
            ch.append((lo, n))
        chunks.append(ch)
        per_tile.append(pt)
    return chunks, per_tile


def _build(S2):
    """Build+schedule the SPMD program. S2 = tuple of (S_low, S_high) per dst tile;
    identical for all 8 cores."""
    S2 = [tuple(s) for s in S2]
    T_L = sum(s[0] for s in S2)
    T_H = sum(s[1] for s in S2)
    TT = T_L + T_H
    chunks, per_tile = _chunk_plan(S2)
    # global subtile index (dloc/norm column): low half at sub_lo, high at T_L + sub_lo
    half_base = (0, T_L)

    nc = bacc.Bacc("TRN2", target_bir_lowering=False, debug=False, num_devices=NCORES,
                   num_swdge_queues=4, dynamic_dma_scratch_size=32768)

    # ---- I/O ----
    TTP = ((TT + MB - 1) // MB) * MB + MB
    xedge_d = nc.dram_tensor("xedge", [P, TT * HID], DT_TAB, kind="ExternalInput")
    gidx_d = nc.dram_tensor("gidx", [P, TT * 8], I16, kind="ExternalInput")
    dloc_d = nc.dram_tensor("dloc", [P, TTP], DT_TAB, kind="ExternalInput")
    dinv_d = nc.dram_tensor("dinv_sl", [P, NT], F32, kind="ExternalInput")
    W_d = [
        nc.dram_tensor("W1", [IN, HID], F32, kind="ExternalInput"),
        nc.dram_tensor("W2", [HID, HID], F32, kind="ExternalInput"),
        nc.dram_tensor("W3", [HID, HID], F32, kind="ExternalInput"),
    ]
    fcW_d = nc.dram_tensor("fcW", [HID, 1], F32, kind="ExternalInput")
    g_d = [nc.dram_tensor("g1", [HID, 1], F32, kind="ExternalInput"),
           nc.dram_tensor("g2", [HID, 1], F32, kind="ExternalInput")]
    bt_d = [nc.dram_tensor("bt1", [HID, 1], F32, kind="ExternalInput"),
            nc.dram_tensor("bt2", [HID, 1], F32, kind="ExternalInput")]
    b3_d = nc.dram_tensor("b3", [HID, 1], F32, kind="ExternalInput")
    outv = nc.dram_tensor("outv", [1, NPC], F32, kind="ExternalOutput")

    with tile.TileContext(nc) as tc:
        with (
            tc.tile_pool(name="meta", bufs=1) as meta,
            tc.tile_pool(name="msgLp", bufs=4) as msgLp,
            tc.tile_pool(name="msgHp", bufs=4) as msgHp,
            tc.tile_pool(name="maskp", bufs=4) as maskp,
            tc.tile_pool(name="zsp", bufs=3) as zsp,
            tc.tile_pool(name="actp", bufs=3) as actp,
            tc.tile_pool(name="hp", bufs=3) as hp,
            tc.tile_pool(name="sqp", bufs=2) as sqp,
            tc.tile_pool(name="zps_p", bufs=2, space="PSUM") as zps_p,
            tc.tile_pool(name="yps_p", bufs=2, space="PSUM") as yps_p,
            tc.tile_pool(name="trps_p", bufs=2, space="PSUM") as trps_p,
            tc.tile_pool(name="fcps_p", bufs=1, space="PSUM") as fcps_p,
            tc.tile_pool(name="dram", bufs=1, space="DRAM") as dram,
        ):
            # ---- resident metadata ----
            gidx_sb = meta.tile([P, TT * 8], I16)
            nc.sync.dma_start(gidx_sb[:], gidx_d[:])
            dloc_sb = meta.tile([P, TTP], DT_TAB)
            nc.sync.dma_start(dloc_sb[:], dloc_d[:])
            dinv_sl = meta.tile([P, NT], F32)
            nc.sync.dma_start(dinv_sl[:], dinv_d[:])
            W_sb = []
            for l in range(3):
                fi = IN if l == 0 else HID
                w = meta.tile([fi, HID], F32, name=f"W{l}_sb")
                nc.sync.dma_start(w[:], W_d[l][:])
                W_sb.append(w)
            fcW_sb = meta.tile([HID, 1], F32)
            nc.sync.dma_start(fcW_sb[:], fcW_d[:])
            g_sb, bt_sb = [], []
            for l in range(2):
                gg = meta.tile([HID, 1], F32, name=f"g{l}_sb")
                nc.sync.dma_start(gg[:], g_d[l][:])
                g_sb.append(gg)
                bb = meta.tile([HID, 1], F32, name=f"bt{l}_sb")
                nc.sync.dma_start(bb[:], bt_d[l][:])
                bt_sb.append(bb)
            b3_sb = meta.tile([HID, 1], F32)
            nc.sync.dma_start(b3_sb[:], b3_d[:])
            eps_sb = meta.tile([P, 1], F32)
            nc.vector.memset(eps_sb[:], BN_EPS)

            ident = meta.tile([P, P], F32)
            make_identity(nc, ident[:])
            iota_i = meta.tile([P, P], mybir.dt.int32)
            nc.gpsimd.iota(iota_i[:], pattern=[[1, P]], base=0, channel_multiplier=0)
            iota_t = meta.tile([P, P], DT_TAB)
            nc.vector.tensor_copy(iota_t[:], iota_i[:])
            iota8 = meta.tile([P, MB * P], DT_TAB)
            for q in range(MB):
                nc.vector.tensor_copy(iota8[:, q * P:(q + 1) * P], iota_t[:])

            # dinv broadcast rows: dinv_bc[:, t*128+j] = dinv of node t*128+j (all partitions)
            dinv_bc = meta.tile([P, NT * P], F32)
            for t in range(NT):
                tr = trps_p.tile([P, P], F32, tag="tr")
                nc.tensor.transpose(tr[:], dinv_sl[:, t:t + 1].to_broadcast([P, P]), ident[:])
                nc.vector.tensor_copy(dinv_bc[:, t * P:(t + 1) * P], tr[:])

            ystore = meta.tile([P, NT * P], F32)
            sums = meta.tile([P, NT], F32)
            sumsq = meta.tile([P, NT], F32)
            out_store = meta.tile([1, NPC], F32)

            # internal DRAM for collectives (fresh per rep: Shared tensors allow one writer)
            def mk_coll(rep):
                tab_in = [dram.tile([NPC, HID], DT_TAB, name=f"tab{l}_in_r{rep}") for l in (1, 2)]
                tab_out = [dram.tile([N, HID], DT_TAB, name=f"tab{l}_out_r{rep}",
                                     addr_space="Shared") for l in (1, 2)]
                st_in = [dram.tile([P, 2], F32, name=f"st{l}_in_r{rep}") for l in (0, 1)]
                st_out = [dram.tile([P, 2], F32, name=f"st{l}_out_r{rep}", addr_space="Shared")
                          for l in (0, 1)]
                return tab_in, tab_out, st_in, st_out

            for _rep in range(REPS):
              tab_in, tab_out, st_in, st_out = mk_coll(_rep)
              for l in range(LAYERS):
                 f_in = IN if l == 0 else HID
                 table = None if l == 0 else tab_out[l - 1]

                 # ---- chunked gathers (issued lazily, consumed tile-major) ----
                 chunk_tiles = {}
                 mask_tiles = {}
                 qrot = [l % 4]

                 def get_chunk(h, cid, l=l, table=table, chunk_tiles=chunk_tiles, qrot=qrot):
                     key = (h, cid)
                     if key not in chunk_tiles:
                         sub_lo, nsub = chunks[h][cid]
                         pool = msgLp if h == 0 else msgHp
                         buf = pool.tile([P, GMAX * HID], DT_TAB, tag=f"msg{h}")
                         g0 = half_base[h] + sub_lo
                         if l == 0:
                             nc.sync.dma_start(buf[:, :nsub * HID],
                                               xedge_d[:, g0 * HID:(g0 + nsub) * HID])
                         else:
                             nidx = nsub * P
                             src_ap = table[0:SPLIT, :] if h == 0 else table[SPLIT:N, :]
                             nc.gpsimd.dma_gather(
                                 buf[:, :nsub * HID].rearrange("p (g e) -> p g e", e=HID),
                                 src_ap,
                                 gidx_sb[:, g0 * 8:(g0 + nsub) * 8],
                                 nidx, nidx, HID, queue_num=qrot[0], single_packet=False,
                             )
                             qrot[0] = (qrot[0] + 1) % 4
                         chunk_tiles[key] = buf
                     return chunk_tiles[key]

                 def get_mask(h, g, mask_tiles=mask_tiles):
                     # batch masks per half so L and H streams never share a tile
                     base = half_base[1] if h else 0
                     q = (g - base) // MB
                     key = (h, q)
                     if key not in mask_tiles:
                         c0 = base + q * MB
                         m8 = maskp.tile([P, MB * P], DT_TAB, tag="mask")
                         nc.vector.tensor_tensor(
                             out=m8[:], in0=iota8[:],
                             in1=dloc_sb[:, c0:c0 + MB].unsqueeze(2)
                                 .to_broadcast([P, MB, P]),
                             op=Alu.is_equal,
                         )
                         mask_tiles[key] = m8
                     r = (g - base) % MB
                     return mask_tiles[key][:, r * P:(r + 1) * P]

                 # ---- aggregation + weight matmul ----
                 for t in range(NT):
                     d_hi = LAST_D if t == NT - 1 else P
                     segs = []
                     for h in (0, 1):
                         sth = S2[t][h]
                         if sth:
                             segs.append((h, per_tile[h][t][0], per_tile[h][t][1], sth))
                     ntot = sum(s[3] for s in segs)
                     zps = zps_p.tile([P, P], F32, tag="zps")
                     k = 0
                     for (h, cid, col0, sth) in segs:
                         buf = get_chunk(h, cid)
                         g0 = half_base[h] + chunks[h][cid][0] + col0
                         for s in range(sth):
                             g = g0 + s
                             mask = get_mask(h, g)
                             nc.tensor.matmul(
                                 zps[:f_in, :],
                                 lhsT=buf[:, (col0 + s) * HID:(col0 + s) * HID + f_in],
                                 rhs=mask,
                                 start=(k == 0), stop=(k == ntot - 1),
                             )
                             k += 1
                     zs = zsp.tile([P, P], F32, tag="zs")
                     nc.vector.tensor_tensor(
                         out=zs[:f_in, :], in0=zps[:f_in, :],
                         in1=dinv_bc[:f_in, t * P:(t + 1) * P], op=Alu.mult,
                     )
                     yps = yps_p.tile([P, P], F32, tag="yps")
                     nc.tensor.matmul(yps[:], lhsT=W_sb[l][:], rhs=zs[:f_in, :],
                                      start=True, stop=True)
                     if l < 2:
                         nc.scalar.activation(
                             out=ystore[:, t * P:t * P + d_hi], in_=yps[:, :d_hi],
                             func=Act.Copy, accum_out=sums[:, t:t + 1],
                         )
                         sq = sqp.tile([P, P], F32, tag="sq")
                         nc.scalar.activation(
                             out=sq[:, :d_hi], in_=yps[:, :d_hi],
                             func=Act.Square, accum_out=sumsq[:, t:t + 1],
                         )
                     else:
                         act3 = actp.tile([P, P], F32, tag="act")
                         nc.scalar.activation(out=act3[:, :d_hi], in_=yps[:, :d_hi],
                                              func=Act.Relu, bias=b3_sb[:], scale=1.0)
                         fcp = fcps_p.tile([1, P], F32, tag="fcp")
                         nc.tensor.matmul(fcp[:1, :d_hi], lhsT=fcW_sb[:], rhs=act3[:, :d_hi],
                                          start=True, stop=True)
                         nc.vector.tensor_copy(out_store[:1, t * P:t * P + d_hi], fcp[:1, :d_hi])

                 if l == LAYERS - 1 and l < 2:
                     nc.vector.tensor_copy(out_store[:1, :], ystore[:1, :NPC])
                     break
                 if l < 2:
                     # ---- BN stats allreduce + coefficients ----
                     stats = meta.tile([P, 2], F32, name=f"stats{l}_r{_rep}")
                     nc.vector.tensor_reduce(stats[:, 0:1], sums[:], axis=mybir.AxisListType.X, op=Alu.add)
                     nc.vector.tensor_reduce(stats[:, 1:2], sumsq[:], axis=mybir.AxisListType.X, op=Alu.add)
                     nc.sync.dma_start(st_in[l][:], stats[:])
                     nc.gpsimd.collective_compute(
                         "AllReduce", Alu.add, replica_groups=[list(range(NCORES))],
                         ins=[st_in[l][:]], outs=[st_out[l][:]],
                     )
                     tot = meta.tile([P, 2], F32, name=f"tot{l}_r{_rep}")
                     nc.sync.dma_start(tot[:], st_out[l][:])
                     cf = meta.tile([P, 6], F32, name=f"cf{l}_r{_rep}")  # mean ex2 var std A B
                     nc.vector.tensor_scalar_mul(cf[:, 0:1], tot[:, 0:1], 1.0 / N)
                     nc.vector.tensor_scalar_mul(cf[:, 1:2], tot[:, 1:2], 1.0 / N)
                     nc.vector.tensor_tensor(out=cf[:, 2:3], in0=cf[:, 0:1], in1=cf[:, 0:1], op=Alu.mult)
                     nc.vector.tensor_tensor(out=cf[:, 2:3], in0=cf[:, 1:2], in1=cf[:, 2:3], op=Alu.subtract)
                     nc.scalar.activation(out=cf[:, 3:4], in_=cf[:, 2:3], func=Act.Sqrt, bias=eps_sb[:], scale=1.0)
                     nc.vector.reciprocal(cf[:, 4:5], cf[:, 3:4])
                     A = meta.tile([P, 1], F32, name=f"A{l}_r{_rep}")
                     B = meta.tile([P, 1], F32, name=f"B{l}_r{_rep}")
                     nc.vector.tensor_tensor(out=A[:], in0=g_sb[l][:], in1=cf[:, 4:5], op=Alu.mult)
                     nc.vector.tensor_tensor(out=cf[:, 5:6], in0=cf[:, 0:1], in1=A[:], op=Alu.mult)
                     nc.vector.tensor_tensor(out=B[:], in0=bt_sb[l][:], in1=cf[:, 5:6], op=Alu.subtract)

                     # ---- epilogue: act, transpose to node-major, store table slice ----
                     for t in range(NT):
                         d_hi = LAST_D if t == NT - 1 else P
                         act = actp.tile([P, P], F32, tag="act")
                         nc.scalar.activation(out=act[:, :d_hi], in_=ystore[:, t * P:t * P + d_hi],
                                              func=Act.Relu, bias=B[:], scale=A[:])
                         tr = trps_p.tile([P, P], F32, tag="tr")
                         nc.tensor.transpose(tr[:d_hi, :], act[:, :d_hi], ident[:])
                         h = hp.tile([P, HID], DT_TAB, tag="h")
                         nc.scalar.activation(out=h[:d_hi, :], in_=tr[:d_hi, :],
                                              func=Act.Copy, scale=dinv_sl[:d_hi, t:t + 1])
                         nc.sync.dma_start(tab_in[l][t * P:t * P + d_hi, :], h[:d_hi, :])
                     nc.gpsimd.collective_compute(
                         "AllGather", Alu.bypass, replica_groups=[list(range(NCORES))],
                         ins=[tab_in[l][:]], outs=[tab_out[l][:]],
                     )

            nc.sync.dma_start(outv[:], out_store[:])

    nc.compile()
    return nc


def _prep(inputs):
    x = np.asarray(inputs["x"], np.float32)
    ei = np.asarray(inputs["edge_index"], np.int64)
    loops = np.arange(N, dtype=np.int64)
    src = np.concatenate([ei[0], loops])
    dst = np.concatenate([ei[1], loops])
    deg = np.bincount(dst, minlength=N).astype(np.float32)
    dinv = (1.0 / np.sqrt(deg)).astype(np.float32)
    xs_pad = np.zeros((N, HID), np.float32)
    xs_pad[:, :IN] = x * dinv[:, None]
    xs_pad = xs_pad.astype(NP_TAB)

    core = dst // NPC
    rem = dst - core * NPC
    tidx = rem >> 7
    loc = (rem & 127).astype(np.float32)
    half = (src >= SPLIT).astype(np.int64)

    order = np.lexsort((half, tidx, core))
    src_s = src[order]
    core_s = core[order]
    tidx_s = tidx[order]
    half_s = half[order]
    loc_s = loc[order]

    gk = (core_s * NT + tidx_s) * 2 + half_s
    cnt = np.bincount(gk, minlength=NCORES * NT * 2).reshape(NCORES, NT, 2)
    S2 = np.ceil(cnt.max(axis=0) / P).astype(np.int64)  # [NT, 2]
    T_L = int(S2[:, 0].sum())
    TT = T_L + int(S2[:, 1].sum())
    offL = np.zeros(NT, np.int64)
    offL[1:] = np.cumsum(S2[:, 0])[:-1]
    offH = np.zeros(NT, np.int64)
    offH[1:] = np.cumsum(S2[:, 1])[:-1]

    starts = np.zeros(NCORES * NT * 2, np.int64)
    starts[1:] = np.cumsum(cnt.reshape(-1))[:-1]
    pos = np.arange(len(src_s)) - starts[gk]
    subl = pos >> 7
    lane = pos & 127
    gsub = np.where(half_s == 0, offL[tidx_s], T_L + offH[tidx_s]) + subl

    TTP = ((TT + 7) // 8) * 8 + 8
    gidx = np.zeros((NCORES, P, TT), np.int16)
    dloc = np.full((NCORES, P, TTP), 1000.0, np.float32)
    gsrc = np.zeros((NCORES, P, TT), np.int32)
    gidx[core_s, lane, gsub] = (src_s - half_s * SPLIT).astype(np.int16)
    dloc[core_s, lane, gsub] = loc_s
    gsrc[core_s, lane, gsub] = src_s

    dinv_pad = np.zeros((NCORES, NT * P), np.float32)
    dinv_pad[:, :NPC] = dinv.reshape(NCORES, NPC)
    dinv_sl = dinv_pad.reshape(NCORES, NT, P).transpose(0, 2, 1).copy()  # [c, P, NT]

    # dma_gather idx layout: global position i=(gsub*128+lane) -> [i%16, i//16],
    # replicated across the 8 groups of 16 partitions.
    idx16 = gidx.transpose(0, 2, 1).reshape(NCORES, TT * 8, 16).transpose(0, 2, 1)
    idx_tile = np.tile(idx16, (1, 8, 1))  # [NCORES, 128, TT*8]

    com = {
        "W1": np.asarray(inputs["W1"], np.float32),
        "W2": np.asarray(inputs["W2"], np.float32),
        "W3": np.asarray(inputs["W3"], np.float32),
        "fcW": np.asarray(inputs["fcW"], np.float32).reshape(HID, 1),
        "g1": np.asarray(inputs["g1"], np.float32).reshape(HID, 1),
        "g2": np.asarray(inputs["g2"], np.float32).reshape(HID, 1),
        "bt1": np.asarray(inputs["bt1"], np.float32).reshape(HID, 1),
        "bt2": np.asarray(inputs["bt2"], np.float32).reshape(HID, 1),
        "b3": np.asarray(inputs["b3"], np.float32).reshape(HID, 1),
    }
    in_maps = []
    for c in range(NCORES):
        m = dict(com)
        m["gidx"] = np.ascontiguousarray(idx_tile[c])
        m["dloc"] = np.ascontiguousarray(dloc[c].astype(NP_TAB))
        m["dinv_sl"] = np.ascontiguousarray(dinv_sl[c])
        # layer-1 messages gathered on host: [P, TT*HID], slot (lane, gsub)
        m["xedge"] = xs_pad[gsrc[c]].reshape(P, TT * HID)
        in_maps.append(m)
    return in_maps, tuple(tuple(int(v) for v in row) for row in S2)


def _get_nc(S2):
    key = (S2, REPS, GMAX, LAYERS)
    if key not in _NC_CACHE:
        _NC_CACHE[key] = _build(S2)
    return _NC_CACHE[key]


class _Exec:
    """jit-once / device_put-once executor mirroring bass2jax.run_bass_via_pjrt."""

    def __init__(self, nc, in_maps):
        import jax
        from jax.sharding import Mesh, PartitionSpec
        from jax.experimental.shard_map import shard_map
        from concourse import bass2jax
        bass2jax.install_neuronx_cc_hook()
        n_cores = NCORES
        part_name = nc.partition_id_tensor.name if nc.partition_id_tensor else None
        in_names, out_names, out_avals, zero_outs = [], [], [], []
        for alloc in nc.m.functions[0].allocations:
            if not isinstance(alloc, mybir.MemoryLocationSet):
                continue
            name = alloc.memorylocations[0].name
            if alloc.kind == "ExternalInput":
                if name != part_name:
                    in_names.append(name)
            elif alloc.kind == "ExternalOutput":
                out_names.append(name)
                shape = tuple(alloc.tensor_shape)
                dtype = mybir.dt.np(alloc.dtype)
                out_avals.append(jax.core.ShapedArray(shape, dtype))
                zero_outs.append(np.zeros(shape, dtype))
        n_params = len(in_names)
        all_names = in_names + out_names
        if part_name is not None:
            all_names = all_names + [part_name]
        self.out_names, self.out_avals, self.n_cores = out_names, out_avals, n_cores

        def _body(*args):
            operands = list(args)
            if part_name is not None:
                operands.append(bass2jax.partition_id_tensor())
            outs = bass2jax._bass_exec_p.bind(
                *operands,
                out_avals=tuple(out_avals),
                in_names=tuple(all_names),
                out_names=tuple(out_names),
                lowering_input_output_aliases=(),
                sim_require_finite=True,
                sim_require_nnan=True,
                nc=nc,
            )
            return tuple(outs)

        devices = jax.devices()[:n_cores]
        mesh = Mesh(np.asarray(devices), ("core",))
        in_specs = (PartitionSpec("core"),) * (n_params + len(out_names))
        out_specs = (PartitionSpec("core"),) * len(out_names)
        self.fn = jax.jit(
            shard_map(_body, mesh=mesh, in_specs=in_specs, out_specs=out_specs,
                      check_rep=False),
            keep_unused=True,
        )
        concat_in = [
            np.concatenate([np.asarray(in_maps[c][k]) for c in range(n_cores)], axis=0)
            for k in in_names
        ]
        concat_zeros = [
            np.zeros((n_cores * z.shape[0], *z.shape[1:]), z.dtype) for z in zero_outs
        ]
        sh = jax.sharding.NamedSharding(mesh, PartitionSpec("core"))
        self.dev_in = [jax.device_put(a, sh) for a in concat_in] + \
                      [jax.device_put(a, sh) for a in concat_zeros]
        for a in self.dev_in:
            a.block_until_ready()

    def run(self):
        outs = self.fn(*self.dev_in)
        for o in outs:
            o.block_until_ready()
        return outs

    def results(self):
        outs = self.run()
        res = [dict() for _ in range(self.n_cores)]
        for i, name in enumerate(self.out_names):
            arr = np.asarray(outs[i]).reshape(self.n_cores, *self.out_avals[i].shape)
            for c in range(self.n_cores):
                res[c][name] = arr[c]
        return res


_EXEC_CACHE = {}


def _get_exec(in_maps, S2):
    key = (S2, REPS, GMAX, LAYERS)
    if key not in _EXEC_CACHE:
        _EXEC_CACHE[key] = _Exec(_get_nc(S2), in_maps)
    return _EXEC_CACHE[key]


def _run(in_maps, S2):
    nc = _get_nc(S2)
    r = bass_utils.run_bass_kernel_spmd(nc, in_maps, core_ids=list(range(NCORES)), trace=False)
    return r


def kernel(**inputs):
    in_maps, S2 = _prep(inputs)
    r = _run(in_maps, S2)
    out = np.concatenate([r.results[c]["outv"].reshape(-1) for c in range(NCORES)])
    fcb = np.asarray(inputs["fcb"], np.float32).reshape(-1)
    out = (out + fcb[0]).astype(np.float32)[:, None]
    # numerically stable sigmoid in fp32
    sig = np.empty_like(out)
    pos = out >= 0
    sig[pos] = 1.0 / (1.0 + np.exp(-out[pos], dtype=np.float32))
    ex = np.exp(out[~pos], dtype=np.float32)
    sig[~pos] = ex / (1.0 + ex)
    return out, sig


# revision 13
# speedup vs baseline: 3.0213x; 1.2954x over previous
"""BindingPocketGNN (3-layer GCN, N=50000, E=800000) on 8 Trainium2 NeuronCores.

Distribution: nodes sharded into 8 contiguous ranges (6250/core). Each core owns the
scatter/aggregation for its destination-node range; edges are routed (host-side) to the
core owning their destination. Source features come from a replicated node-major table
(input x for layer 1; AllGather-replicated activations for layers 2/3).

v2: the per-edge gather uses a few big gpsimd dma_gather (InstDMAGatherAnt) calls per
layer (0.34ns/descriptor batched SWDGE) instead of ~900 small indirect_dma_start ops
(~1us fixed each, which made the baseline SWDGE/gpsimd-bound at 87% occupancy).
dma_gather takes int16 indices, so the 50000-row table is addressed as two halves
(src < 32768 via table[0:32768], src >= 32768 via table[32768:]); edges are grouped
host-side by (dst tile, half) into 128-edge subtiles, padded with idx=0 slots that a
one-hot mask zeroes out.

Per layer, on each core (feat-major formulation so BN/bias are per-partition):
    msg[e, f]     = table[src_e, f]                      (chunked dma_gather, bf16)
    mask[e, d]    = (iota[d]==dloc[e]) * norm_e          (DVE tensor_scalar, one op)
    z^T[f, d]    += msg^T @ mask                         (TensorE accumulate over subtiles)
    y^T[f_out, d] = W^T @ z^T                            (TensorE)
    stats         = AllReduce(sum/sumsq of y)            (1KB collective; BN layers)
    act^T         = Relu(A*y^T + B)                      (ScalarE, per-partition A/B)
    h             = act^T transposed to node-major       (TensorE transpose)
    table_{l+1}   = AllGather(h)                         (collective; layers 1,2)
Layer 3 feeds a [128->1] FC matmul; +fcb and sigmoid applied on host.

Tables are prescaled by dinv[src] (host for x, ACT-scale in the epilogue for h);
dinv[dst] is applied per dst tile via a broadcast row table (dinv_bc). Masks are pure
0/1 bf16 built with tensor_tensor is_equal (per-partition-scalar AP operands on DVE
cost ~1us/op in scalar-fetch mode; the broadcast tensor_tensor form does not).
"""
import sys
if "/opt/trn_rl_repo" not in sys.path:
    sys.path.insert(0, "/opt/trn_rl_repo")

import os
import numpy as np
import ml_dtypes

import concourse.bass as bass
import concourse.bacc as bacc
import concourse.mybir as mybir
import concourse.tile as tile
from concourse import bass_utils
from concourse.masks import make_identity

N = 50000
E = 800000
IN, HID = 64, 128
BN_EPS = 1e-5
NCORES = 8
NPC = N // NCORES          # 6250 nodes per core
P = 128
NT = (NPC + P - 1) // P    # 49 dst tiles per core
LAST_D = NPC - (NT - 1) * P  # 106
HNPC = NPC // 2            # 3125: half-slice rows per core (AllGather pipelining)

F32 = mybir.dt.float32
I16 = mybir.dt.int16
BF16 = mybir.dt.bfloat16
DT_TAB = BF16
NP_TAB = np.dtype(ml_dtypes.bfloat16)

REPS = int(os.environ.get("GCN_REPS", "1"))
GMAX = int(os.environ.get("GCN_GMAX", "24"))   # max subtiles per dma_gather chunk
LAYERS = int(os.environ.get("GCN_LAYERS", "3"))  # debug: run only first k layers
MB = 8                                           # mask-build batch (subtiles per DVE op)

Alu = mybir.AluOpType
Act = mybir.ActivationFunctionType

_NC_CACHE = {}


def _chunk_plan(S2):
    """Greedy-pack tiles' subtile groups into gather chunks of <= GMAX subtiles.
    Returns (chunks, per_tile): chunks[h] = list of (sub_lo_in_half, n_sub);
    per_tile[h][t] = (chunk_id, col0) locating tile t's first subtile of half h."""
    chunks, per_tile = [], []
    for h in (0, 1):
        ch, pt = [], []
        lo, n = 0, 0
        for t in range(NT):
            s = S2[t][h]
            if n + s > GMAX and n > 0:
                ch.append((lo, n))
                lo += n
                n = 0
            pt.append((len(ch), n))
            n += s
        if n:
            ch.append((lo, n))
        chunks.append(ch)
        per_tile.append(pt)
    return chunks, per_tile


def _build(S2):
    """Build+schedule the SPMD program. S2 = tuple of (S_low, S_high) per dst tile;
    identical for all 8 cores."""
    S2 = [tuple(s) for s in S2]
    T_L = sum(s[0] for s in S2)
    T_H = sum(s[1] for s in S2)
    TT = T_L + T_H
    chunks, per_tile = _chunk_plan(S2)
    # global subtile index (dloc/norm column): low half at sub_lo, high at T_L + sub_lo
    half_base = (0, T_L)

    nc = bacc.Bacc("TRN2", target_bir_lowering=False, debug=False, num_devices=NCORES,
                   num_swdge_queues=4, dynamic_dma_scratch_size=32768)

    # ---- I/O ----
    TTP = ((TT + MB - 1) // MB) * MB + MB
    xedge_d = nc.dram_tensor("xedge", [P, TT * HID], DT_TAB, kind="ExternalInput")
    gidx_d = nc.dram_tensor("gidx", [P, TT * 8], I16, kind="ExternalInput")
    dloc_d = nc.dram_tensor("dloc", [P, TTP], DT_TAB, kind="ExternalInput")
    dinv_d = nc.dram_tensor("dinv_sl", [P, NT], F32, kind="ExternalInput")
    W_d = [
        nc.dram_tensor("W1", [IN, HID], F32, kind="ExternalInput"),
        nc.dram_tensor("W2", [HID, HID], F32, kind="ExternalInput"),
        nc.dram_tensor("W3", [HID, HID], F32, kind="ExternalInput"),
    ]
    fcW_d = nc.dram_tensor("fcW", [HID, 1], F32, kind="ExternalInput")
    g_d = [nc.dram_tensor("g1", [HID, 1], F32, kind="ExternalInput"),
           nc.dram_tensor("g2", [HID, 1], F32, kind="ExternalInput")]
    bt_d = [nc.dram_tensor("bt1", [HID, 1], F32, kind="ExternalInput"),
            nc.dram_tensor("bt2", [HID, 1], F32, kind="ExternalInput")]
    b3_d = nc.dram_tensor("b3", [HID, 1], F32, kind="ExternalInput")
    outv = nc.dram_tensor("outv", [1, NPC], F32, kind="ExternalOutput")

    with tile.TileContext(nc) as tc:
        with (
            tc.tile_pool(name="meta", bufs=1) as meta,
            tc.tile_pool(name="msgLp", bufs=5) as msgLp,
            tc.tile_pool(name="msgHp", bufs=5) as msgHp,
            tc.tile_pool(name="maskp", bufs=6) as maskp,
            tc.tile_pool(name="zsp", bufs=3) as zsp,
            tc.tile_pool(name="actp", bufs=3) as actp,
            tc.tile_pool(name="hp", bufs=3) as hp,
            tc.tile_pool(name="sqp", bufs=2) as sqp,
            tc.tile_pool(name="zps_p", bufs=3, space="PSUM") as zps_p,
            tc.tile_pool(name="yps_p", bufs=2, space="PSUM") as yps_p,
            tc.tile_pool(name="trps_p", bufs=2, space="PSUM") as trps_p,
            tc.tile_pool(name="fcps_p", bufs=1, space="PSUM") as fcps_p,
            tc.tile_pool(name="dram", bufs=1, space="DRAM") as dram,
        ):
            # ---- resident metadata ----
            gidx_sb = meta.tile([P, TT * 8], I16)
            nc.sync.dma_start(gidx_sb[:], gidx_d[:])
            dloc_sb = meta.tile([P, TTP], DT_TAB)
            nc.sync.dma_start(dloc_sb[:], dloc_d[:])
            dinv_sl = meta.tile([P, NT], F32)
            nc.sync.dma_start(dinv_sl[:], dinv_d[:])
            W_sb = []
            for l in range(3):
                fi = IN if l == 0 else HID
                w = meta.tile([fi, HID], F32, name=f"W{l}_sb")
                nc.sync.dma_start(w[:], W_d[l][:])
                W_sb.append(w)
            fcW_sb = meta.tile([HID, 1], F32)
            nc.sync.dma_start(fcW_sb[:], fcW_d[:])
            g_sb, bt_sb = [], []
            for l in range(2):
                gg = meta.tile([HID, 1], F32, name=f"g{l}_sb")
                nc.sync.dma_start(gg[:], g_d[l][:])
                g_sb.append(gg)
                bb = meta.tile([HID, 1], F32, name=f"bt{l}_sb")
                nc.sync.dma_start(bb[:], bt_d[l][:])
                bt_sb.append(bb)
            b3_sb = meta.tile([HID, 1], F32)
            nc.sync.dma_start(b3_sb[:], b3_d[:])
            eps_sb = meta.tile([P, 1], F32)
            nc.vector.memset(eps_sb[:], BN_EPS)

            ident = meta.tile([P, P], F32)
            make_identity(nc, ident[:])
            iota_i = meta.tile([P, P], mybir.dt.int32)
            nc.gpsimd.iota(iota_i[:], pattern=[[1, P]], base=0, channel_multiplier=0)
            iota_t = meta.tile([P, P], DT_TAB)
            nc.vector.tensor_copy(iota_t[:], iota_i[:])
            iota8 = meta.tile([P, MB * P], DT_TAB)
            for q in range(MB):
                nc.vector.tensor_copy(iota8[:, q * P:(q + 1) * P], iota_t[:])

            # ncfw warmup: a dummy collective pays first-collective setup cost
            # while the metadata DMAs / dinv_bc build run
            wu_in = dram.tile([P, 2], F32, name="wu_in")
            wu_out = dram.tile([P, 2], F32, name="wu_out", addr_space="Shared")
            wu_sb = meta.tile([P, 2], F32)
            nc.vector.memset(wu_sb[:], 0.0)
            nc.sync.dma_start(wu_in[:], wu_sb[:])
            nc.gpsimd.collective_compute(
                "AllReduce", Alu.add, replica_groups=[list(range(NCORES))],
                ins=[wu_in[:]], outs=[wu_out[:]],
            )

            # dinv broadcast rows: dinv_bc[:, t*128+j] = dinv of node t*128+j (all partitions)
            dinv_bc = meta.tile([P, NT * P], F32)
            for t in range(NT):
                tr = trps_p.tile([P, P], F32, tag="tr")
                nc.tensor.transpose(tr[:], dinv_sl[:, t:t + 1].to_broadcast([P, P]), ident[:])
                nc.vector.tensor_copy(dinv_bc[:, t * P:(t + 1) * P], tr[:])

            ystore = meta.tile([P, NT * P], F32)
            sums = meta.tile([P, NT], F32)
            sumsq = meta.tile([P, NT], F32)
            out_store = meta.tile([1, NPC], F32)

            # internal DRAM for collectives (fresh per rep: Shared tensors allow one writer)
            def mk_coll(rep):
                tab_in = [[dram.tile([HNPC, HID], DT_TAB, name=f"tab{l}{ab}_in_r{rep}")
                           for ab in "ab"] for l in (1, 2)]
                tab_out = [[dram.tile([NCORES * HNPC, HID], DT_TAB,
                                      name=f"tab{l}{ab}_out_r{rep}", addr_space="Shared")
                            for ab in "ab"] for l in (1, 2)]
                st_in = [dram.tile([P, 2], F32, name=f"st{l}_in_r{rep}") for l in (0, 1)]
                st_out = [dram.tile([P, 2], F32, name=f"st{l}_out_r{rep}", addr_space="Shared")
                          for l in (0, 1)]
                return tab_in, tab_out, st_in, st_out

            for _rep in range(REPS):
              tab_in, tab_out, st_in, st_out = mk_coll(_rep)
              for l in range(LAYERS):
                 f_in = IN if l == 0 else HID
                 table = None if l == 0 else tab_out[l - 1]

                 # ---- chunked gathers (issued lazily, consumed tile-major) ----
                 chunk_tiles = {}
                 mask_tiles = {}
                 qrot = [l % 4]

                 def get_chunk(h, cid, l=l, table=table, chunk_tiles=chunk_tiles, qrot=qrot):
                     key = (h, cid)
                     if key not in chunk_tiles:
                         sub_lo, nsub = chunks[h][cid]
                         pool = msgLp if h == 0 else msgHp
                         buf = pool.tile([P, GMAX * HID], DT_TAB, tag=f"msg{h}")
                         g0 = half_base[h] + sub_lo
                         if l == 0:
                             nc.sync.dma_start(buf[:, :nsub * HID],
                                               xedge_d[:, g0 * HID:(g0 + nsub) * HID])
                         else:
                             nidx = nsub * P
                             src_ap = table[h][:, :]
                             nc.gpsimd.dma_gather(
                                 buf[:, :nsub * HID].rearrange("p (g e) -> p g e", e=HID),
                                 src_ap,
                                 gidx_sb[:, g0 * 8:(g0 + nsub) * 8],
                                 nidx, nidx, HID, queue_num=qrot[0], single_packet=False,
                             )
                             qrot[0] = (qrot[0] + 1) % 4
                         chunk_tiles[key] = buf
                     return chunk_tiles[key]

                 def get_mask(h, g, mask_tiles=mask_tiles):
                     # batch masks per half so L and H streams never share a tile
                     base = half_base[1] if h else 0
                     q = (g - base) // MB
                     key = (h, q)
                     if key not in mask_tiles:
                         c0 = base + q * MB
                         m8 = maskp.tile([P, MB * P], DT_TAB, tag="mask")
                         nc.vector.tensor_tensor(
                             out=m8[:], in0=iota8[:],
                             in1=dloc_sb[:, c0:c0 + MB].unsqueeze(2)
                                 .to_broadcast([P, MB, P]),
                             op=Alu.is_equal,
                         )
                         mask_tiles[key] = m8
                     r = (g - base) % MB
                     return mask_tiles[key][:, r * P:(r + 1) * P]

                 # ---- aggregation + weight matmul ----
                 for t in range(NT):
                     d_hi = LAST_D if t == NT - 1 else P
                     segs = []
                     for h in (0, 1):
                         sth = S2[t][h]
                         if sth:
                             segs.append((h, per_tile[h][t][0], per_tile[h][t][1], sth))
                     ntot = sum(s[3] for s in segs)
                     zps = zps_p.tile([P, P], F32, tag="zps")
                     k = 0
                     for (h, cid, col0, sth) in segs:
                         buf = get_chunk(h, cid)
                         g0 = half_base[h] + chunks[h][cid][0] + col0
                         for s in range(sth):
                             g = g0 + s
                             mask = get_mask(h, g)
                             nc.tensor.matmul(
                                 zps[:f_in, :],
                                 lhsT=buf[:, (col0 + s) * HID:(col0 + s) * HID + f_in],
                                 rhs=mask,
                                 start=(k == 0), stop=(k == ntot - 1),
                             )
                             k += 1
                     zs = zsp.tile([P, P], F32, tag="zs")
                     nc.vector.tensor_tensor(
                         out=zs[:f_in, :], in0=zps[:f_in, :],
                         in1=dinv_bc[:f_in, t * P:(t + 1) * P], op=Alu.mult,
                     )
                     yps = yps_p.tile([P, P], F32, tag="yps")
                     nc.tensor.matmul(yps[:], lhsT=W_sb[l][:], rhs=zs[:f_in, :],
                                      start=True, stop=True)
                     if l < 2:
                         nc.scalar.activation(
                             out=ystore[:, t * P:t * P + d_hi], in_=yps[:, :d_hi],
                             func=Act.Copy, accum_out=sums[:, t:t + 1],
                         )
                         sq = sqp.tile([P, P], F32, tag="sq")
                         nc.scalar.activation(
                             out=sq[:, :d_hi], in_=yps[:, :d_hi],
                             func=Act.Square, accum_out=sumsq[:, t:t + 1],
                         )
                     else:
                         act3 = actp.tile([P, P], F32, tag="act")
                         nc.scalar.activation(out=act3[:, :d_hi], in_=yps[:, :d_hi],
                                              func=Act.Relu, bias=b3_sb[:], scale=1.0)
                         fcp = fcps_p.tile([1, P], F32, tag="fcp")
                         nc.tensor.matmul(fcp[:1, :d_hi], lhsT=fcW_sb[:], rhs=act3[:, :d_hi],
                                          start=True, stop=True)
                         nc.vector.tensor_copy(out_store[:1, t * P:t * P + d_hi], fcp[:1, :d_hi])

                 if l == LAYERS - 1 and l < 2:
                     nc.vector.tensor_copy(out_store[:1, :], ystore[:1, :NPC])
                     break
                 if l < 2:
                     # ---- BN stats allreduce + coefficients ----
                     stats = meta.tile([P, 2], F32, name=f"stats{l}_r{_rep}")
                     nc.vector.tensor_reduce(stats[:, 0:1], sums[:], axis=mybir.AxisListType.X, op=Alu.add)
                     nc.vector.tensor_reduce(stats[:, 1:2], sumsq[:], axis=mybir.AxisListType.X, op=Alu.add)
                     nc.sync.dma_start(st_in[l][:], stats[:])
                     nc.gpsimd.collective_compute(
                         "AllReduce", Alu.add, replica_groups=[list(range(NCORES))],
                         ins=[st_in[l][:]], outs=[st_out[l][:]],
                     )
                     tot = meta.tile([P, 2], F32, name=f"tot{l}_r{_rep}")
                     nc.sync.dma_start(tot[:], st_out[l][:])
                     cf = meta.tile([P, 6], F32, name=f"cf{l}_r{_rep}")  # mean ex2 var std A B
                     nc.vector.tensor_scalar_mul(cf[:, 0:1], tot[:, 0:1], 1.0 / N)
                     nc.vector.tensor_scalar_mul(cf[:, 1:2], tot[:, 1:2], 1.0 / N)
                     nc.vector.tensor_tensor(out=cf[:, 2:3], in0=cf[:, 0:1], in1=cf[:, 0:1], op=Alu.mult)
                     nc.vector.tensor_tensor(out=cf[:, 2:3], in0=cf[:, 1:2], in1=cf[:, 2:3], op=Alu.subtract)
                     nc.scalar.activation(out=cf[:, 3:4], in_=cf[:, 2:3], func=Act.Sqrt, bias=eps_sb[:], scale=1.0)
                     nc.vector.reciprocal(cf[:, 4:5], cf[:, 3:4])
                     A = meta.tile([P, 1], F32, name=f"A{l}_r{_rep}")
                     B = meta.tile([P, 1], F32, name=f"B{l}_r{_rep}")
                     nc.vector.tensor_tensor(out=A[:], in0=g_sb[l][:], in1=cf[:, 4:5], op=Alu.mult)
                     nc.vector.tensor_tensor(out=cf[:, 5:6], in0=cf[:, 0:1], in1=A[:], op=Alu.mult)
                     nc.vector.tensor_tensor(out=B[:], in0=bt_sb[l][:], in1=cf[:, 5:6], op=Alu.subtract)

                     # ---- epilogue: act, transpose to node-major, store table slice ----
                     for t in range(NT):
                         d_hi = LAST_D if t == NT - 1 else P
                         act = actp.tile([P, P], F32, tag="act")
                         nc.scalar.activation(out=act[:, :d_hi], in_=ystore[:, t * P:t * P + d_hi],
                                              func=Act.Relu, bias=B[:], scale=A[:])
                         tr = trps_p.tile([P, P], F32, tag="tr")
                         nc.tensor.transpose(tr[:d_hi, :], act[:, :d_hi], ident[:])
                         h = hp.tile([P, HID], DT_TAB, tag="h")
                         nc.scalar.activation(out=h[:d_hi, :], in_=tr[:d_hi, :],
                                              func=Act.Copy, scale=dinv_sl[:d_hi, t:t + 1])
                         r0, r1 = t * P, t * P + d_hi
                         if r1 <= HNPC:
                             nc.sync.dma_start(tab_in[l][0][r0:r1, :], h[:d_hi, :])
                         elif r0 >= HNPC:
                             nc.sync.dma_start(tab_in[l][1][r0 - HNPC:r1 - HNPC, :], h[:d_hi, :])
                         else:
                             k = HNPC - r0
                             nc.sync.dma_start(tab_in[l][0][r0:HNPC, :], h[:k, :])
                             nc.sync.dma_start(tab_in[l][1][0:r1 - HNPC, :], h[k:d_hi, :])
                     for ab in (0, 1):
                         nc.gpsimd.collective_compute(
                             "AllGather", Alu.bypass, replica_groups=[list(range(NCORES))],
                             ins=[tab_in[l][ab][:]], outs=[tab_out[l][ab][:]],
                         )

            nc.sync.dma_start(outv[:], out_store[:])

    nc.compile()
    return nc


def _prep(inputs):
    x = np.asarray(inputs["x"], np.float32)
    ei = np.asarray(inputs["edge_index"], np.int64)
    loops = np.arange(N, dtype=np.int64)
    src = np.concatenate([ei[0], loops])
    dst = np.concatenate([ei[1], loops])
    deg = np.bincount(dst, minlength=N).astype(np.float32)
    dinv = (1.0 / np.sqrt(deg)).astype(np.float32)
    xs_pad = np.zeros((N, HID), np.float32)
    xs_pad[:, :IN] = x * dinv[:, None]
    xs_pad = xs_pad.astype(NP_TAB)

    core = dst // NPC
    rem = dst - core * NPC
    tidx = rem >> 7
    loc = (rem & 127).astype(np.float32)
    # half = which half-table (AllGather half) the source row lives in
    score = src // NPC
    srem = src - score * NPC
    half = (srem >= HNPC).astype(np.int64)
    sidx = score * HNPC + srem - half * HNPC   # row within half-table, < 25000

    order = np.lexsort((half, tidx, core))
    src_s = src[order]
    sidx_s = sidx[order]
    core_s = core[order]
    tidx_s = tidx[order]
    half_s = half[order]
    loc_s = loc[order]

    gk = (core_s * NT + tidx_s) * 2 + half_s
    cnt = np.bincount(gk, minlength=NCORES * NT * 2).reshape(NCORES, NT, 2)
    S2 = np.ceil(cnt.max(axis=0) / P).astype(np.int64)  # [NT, 2]
    T_L = int(S2[:, 0].sum())
    TT = T_L + int(S2[:, 1].sum())
    offL = np.zeros(NT, np.int64)
    offL[1:] = np.cumsum(S2[:, 0])[:-1]
    offH = np.zeros(NT, np.int64)
    offH[1:] = np.cumsum(S2[:, 1])[:-1]

    starts = np.zeros(NCORES * NT * 2, np.int64)
    starts[1:] = np.cumsum(cnt.reshape(-1))[:-1]
    pos = np.arange(len(src_s)) - starts[gk]
    subl = pos >> 7
    lane = pos & 127
    gsub = np.where(half_s == 0, offL[tidx_s], T_L + offH[tidx_s]) + subl

    TTP = ((TT + 7) // 8) * 8 + 8
    gidx = np.zeros((NCORES, P, TT), np.int16)
    dloc = np.full((NCORES, P, TTP), 1000.0, np.float32)
    gsrc = np.zeros((NCORES, P, TT), np.int32)
    gidx[core_s, lane, gsub] = sidx_s.astype(np.int16)
    dloc[core_s, lane, gsub] = loc_s
    gsrc[core_s, lane, gsub] = src_s

    dinv_pad = np.zeros((NCORES, NT * P), np.float32)
    dinv_pad[:, :NPC] = dinv.reshape(NCORES, NPC)
    dinv_sl = dinv_pad.reshape(NCORES, NT, P).transpose(0, 2, 1).copy()  # [c, P, NT]

    # dma_gather idx layout: global position i=(gsub*128+lane) -> [i%16, i//16],
    # replicated across the 8 groups of 16 partitions.
    idx16 = gidx.transpose(0, 2, 1).reshape(NCORES, TT * 8, 16).transpose(0, 2, 1)
    idx_tile = np.tile(idx16, (1, 8, 1))  # [NCORES, 128, TT*8]

    com = {
        "W1": np.asarray(inputs["W1"], np.float32),
        "W2": np.asarray(inputs["W2"], np.float32),
        "W3": np.asarray(inputs["W3"], np.float32),
        "fcW": np.asarray(inputs["fcW"], np.float32).reshape(HID, 1),
        "g1": np.asarray(inputs["g1"], np.float32).reshape(HID, 1),
        "g2": np.asarray(inputs["g2"], np.float32).reshape(HID, 1),
        "bt1": np.asarray(inputs["bt1"], np.float32).reshape(HID, 1),
        "bt2": np.asarray(inputs["bt2"], np.float32).reshape(HID, 1),
        "b3": np.asarray(inputs["b3"], np.float32).reshape(HID, 1),
    }
    in_maps = []
    for c in range(NCORES):
        m = dict(com)
        m["gidx"] = np.ascontiguousarray(idx_tile[c])
        m["dloc"] = np.ascontiguousarray(dloc[c].astype(NP_TAB))
        m["dinv_sl"] = np.ascontiguousarray(dinv_sl[c])
        # layer-1 messages gathered on host: [P, TT*HID], slot (lane, gsub)
        m["xedge"] = xs_pad[gsrc[c]].reshape(P, TT * HID)
        in_maps.append(m)
    return in_maps, tuple(tuple(int(v) for v in row) for row in S2)


def _get_nc(S2):
    key = (S2, REPS, GMAX, LAYERS)
    if key not in _NC_CACHE:
        _NC_CACHE[key] = _build(S2)
    return _NC_CACHE[key]


class _Exec:
    """jit-once / device_put-once executor mirroring bass2jax.run_bass_via_pjrt."""

    def __init__(self, nc, in_maps):
        import jax
        from jax.sharding import Mesh, PartitionSpec
        from jax.experimental.shard_map import shard_map
        from concourse import bass2jax
        bass2jax.install_neuronx_cc_hook()
        n_cores = NCORES
        part_name = nc.partition_id_tensor.name if nc.partition_id_tensor else None
        in_names, out_names, out_avals, zero_outs = [], [], [], []
        for alloc in nc.m.functions[0].allocations:
            if not isinstance(alloc, mybir.MemoryLocationSet):
                continue
            name = alloc.memorylocations[0].name
            if alloc.kind == "ExternalInput":
                if name != part_name:
                    in_names.append(name)
            elif alloc.kind == "ExternalOutput":
                out_names.append(name)
                shape = tuple(alloc.tensor_shape)
                dtype = mybir.dt.np(alloc.dtype)
                out_avals.append(jax.core.ShapedArray(shape, dtype))
                zero_outs.append(np.zeros(shape, dtype))
        n_params = len(in_names)
        all_names = in_names + out_names
        if part_name is not None:
            all_names = all_names + [part_name]
        self.out_names, self.out_avals, self.n_cores = out_names, out_avals, n_cores

        def _body(*args):
            operands = list(args)
            if part_name is not None:
                operands.append(bass2jax.partition_id_tensor())
            outs = bass2jax._bass_exec_p.bind(
                *operands,
                out_avals=tuple(out_avals),
                in_names=tuple(all_names),
                out_names=tuple(out_names),
                lowering_input_output_aliases=(),
                sim_require_finite=True,
                sim_require_nnan=True,
                nc=nc,
            )
            return tuple(outs)

        devices = jax.devices()[:n_cores]
        mesh = Mesh(np.asarray(devices), ("core",))
        in_specs = (PartitionSpec("core"),) * (n_params + len(out_names))
        out_specs = (PartitionSpec("core"),) * len(out_names)
        self.fn = jax.jit(
            shard_map(_body, mesh=mesh, in_specs=in_specs, out_specs=out_specs,
                      check_rep=False),
            keep_unused=True,
        )
        concat_in = [
            np.concatenate([np.asarray(in_maps[c][k]) for c in range(n_cores)], axis=0)
            for k in in_names
        ]
        concat_zeros = [
            np.zeros((n_cores * z.shape[0], *z.shape[1:]), z.dtype) for z in zero_outs
        ]
        sh = jax.sharding.NamedSharding(mesh, PartitionSpec("core"))
        self.dev_in = [jax.device_put(a, sh) for a in concat_in] + \
                      [jax.device_put(a, sh) for a in concat_zeros]
        for a in self.dev_in:
            a.block_until_ready()

    def run(self):
        outs = self.fn(*self.dev_in)
        for o in outs:
            o.block_until_ready()
        return outs

    def results(self):
        outs = self.run()
        res = [dict() for _ in range(self.n_cores)]
        for i, name in enumerate(self.out_names):
            arr = np.asarray(outs[i]).reshape(self.n_cores, *self.out_avals[i].shape)
            for c in range(self.n_cores):
                res[c][name] = arr[c]
        return res


_EXEC_CACHE = {}


def _get_exec(in_maps, S2):
    key = (S2, REPS, GMAX, LAYERS)
    if key not in _EXEC_CACHE:
        _EXEC_CACHE[key] = _Exec(_get_nc(S2), in_maps)
    return _EXEC_CACHE[key]


def _run(in_maps, S2):
    nc = _get_nc(S2)
    r = bass_utils.run_bass_kernel_spmd(nc, in_maps, core_ids=list(range(NCORES)), trace=False)
    return r


def kernel(**inputs):
    in_maps, S2 = _prep(inputs)
    r = _run(in_maps, S2)
    out = np.concatenate([r.results[c]["outv"].reshape(-1) for c in range(NCORES)])
    fcb = np.asarray(inputs["fcb"], np.float32).reshape(-1)
    out = (out + fcb[0]).astype(np.float32)[:, None]
    # numerically stable sigmoid in fp32
    sig = np.empty_like(out)
    pos = out >= 0
    sig[pos] = 1.0 / (1.0 + np.exp(-out[pos], dtype=np.float32))
    ex = np.exp(out[~pos], dtype=np.float32)
    sig[~pos] = ex / (1.0 + ex)
    return out, sig
